# revision 1
# baseline (speedup 1.0000x reference)
"""Multi-head attention kernel for Trainium2, sharded over 8 NeuronCores.

Sharding: data parallel over batch (B=2 -> 4 cores each) x tensor parallel
over heads (12 heads -> 3 heads per core). Each core computes QKV projections,
attention, and a partial output projection for its 3 heads; the per-head
partial output projections are summed on the host (the all-reduce of the
tensor-parallel hint, done during the gather step) and the output bias added.

Layout choices (per core):
  - x arrives pre-transposed and pre-tiled as xT [128, 6, 2048] so the
    contraction dim (d) sits on SBUF partitions for all QKV matmuls with no
    on-device transpose and fully-contiguous DMA.
  - q and k are produced head-by-head directly in transposed form via a
    stacked weight [Wq_h | Wk_h]; scores are computed transposed
    (scoresT [s_k, s_q]) so the softmax probabilities feed the probs@V
    matmul with no transpose.
  - softmax denominators ride for free: the V operand tile carries a block of
    ones columns, so rows 64..127 of the probs@V accumulation are the
    per-query sums of exp(scores); division is a cheap [64, SQ]
    reciprocal+multiply on the context, not on the S x S probabilities.
  - matmuls run in float32r (fp32 storage, reduced-precision multiply, 4x
    the fp32 matmul rate) except probs@V which runs in fp16 (probs are
    positive and < exp(6), well inside fp16 range, and fp16 keeps 10
    mantissa bits vs bf16's 7; the exp output is written as fp16 directly
    by the scalar engine).
  - emission order hand-weaves independent PE work (v/qk/out projections)
    into the ACT-paced scores->exp->probs@V pipeline so the PE stream never
    idles waiting on exp results.
"""

from collections import deque

import numpy as np

import concourse.mybir as mybir
from concourse import bacc
from concourse.tile import TileContext
from concourse.bass_utils import run_bass_kernel_spmd

H, D, DH = 12, 768, 64
B, S = 2, 2048
NCORES = 8
CORES_PER_BATCH = 4
HPC = 3  # heads per core
SQ = 512  # query-chunk width
NSQ = S // SQ  # 4
NSK = S // 128  # 16 key chunks
NDC = D // 128  # 6 contraction chunks

F32 = mybir.dt.float32
F32R = mybir.dt.float32r
F16 = mybir.dt.float16
ADD = mybir.AluOpType.add
MULT = mybir.AluOpType.mult
EXP = mybir.ActivationFunctionType.Exp


def _build_module():
    nc = bacc.Bacc("TRN2", target_bir_lowering=False, debug=False, num_devices=NCORES)
    xT = nc.declare_dram_parameter("xT", [128, NDC, S], F32R, isOutput=False)
    wqk = nc.declare_dram_parameter("wqk", [128, HPC, NDC, 128], F32R, isOutput=False)
    wv = nc.declare_dram_parameter("wv", [128, NDC, 256], F32R, isOutput=False)
    wo01 = nc.declare_dram_parameter("wo01", [128, D], F32R, isOutput=False)
    wo2 = nc.declare_dram_parameter("wo2", [64, D], F32R, isOutput=False)
    bqk = nc.declare_dram_parameter("bqk", [128, HPC], F32, isOutput=False)
    bv = nc.declare_dram_parameter("bv", [128, HPC * DH], F32, isOutput=False)
    out = nc.declare_dram_parameter("out", [S, D], F32, isOutput=True)

    with TileContext(nc) as tc:
        _body(nc, tc, xT, wqk, wv, wo01, wo2, bqk, bv, out)
    nc.compile()
    return nc


def _body(nc, tc, xT, wqk, wv, wo01, wo2, bqk, bv, out):
    with (
        tc.tile_pool(name="persist", bufs=1) as P1,
        tc.tile_pool(name="work", bufs=4) as W2,
        tc.tile_pool(name="probs", bufs=2) as PR,
        # PSUM budget is 8 banks of [128, 512] fp32:
        #   ACC: one shared rotating pool for qk-proj, v-proj, ctx accum and
        #        out-proj tiles (4 banks)
        #   SPS: [128, 1024] score tiles, double-buffered (4 banks) — pairs of
        #        key-chunks share one tile so exp runs 1024 wide
        tc.tile_pool(name="acc", bufs=4, space="PSUM") as ACC,
        tc.tile_pool(name="sps", bufs=2, space="PSUM") as SPS,
    ):
        xT_sb = P1.tile([128, NDC, S], F32R, tag="xT")
        wqk_sb = P1.tile([128, HPC, NDC, 128], F32R, tag="wqk")
        wv_sb = P1.tile([128, NDC, 256], F32R, tag="wv")
        wo01_sb = P1.tile([128, D], F32R, tag="wo01")
        wo2_sb = P1.tile([64, D], F32R, tag="wo2")
        bqk_sb = P1.tile([128, HPC], F32, tag="bqk")
        bv_sb = P1.tile([128, HPC * DH], F32, tag="bv")
        # q/k transposed per head (separate tiles: matmul operands must share
        # the SBUF base partition, so both live at partitions 0..63)
        qT = [
            P1.tile([64, S], F32R, tag=f"qT{h}", name=f"qT{h}")
            for h in range(HPC)
        ]
        kT = [
            P1.tile([64, S], F32R, tag=f"kT{h}", name=f"kT{h}")
            for h in range(HPC)
        ]
        # v tiles: per key-chunk, per head a [v_h (64) | ones (64)] block
        vp = P1.tile([128, NSK, HPC * 128], F16, tag="vp")

        # DMAs in first-needed order. Each dma_start pays ~0.6-1us of
        # serialized DGE overhead, so batch big — except the first matmul's
        # dependencies (wqk head 0, xT query-chunk 0), which are split fine
        # so PE can start within ~1us.
        nc.sync.dma_start(xT_sb[:, 0, 0:SQ], xT[:, 0, 0:SQ])
        nc.sync.dma_start(wqk_sb[:, 0, :, :], wqk[:, 0, :, :])
        for o in range(1, NDC):
            nc.sync.dma_start(xT_sb[:, o, 0:SQ], xT[:, o, 0:SQ])
        nc.sync.dma_start(bqk_sb[:], bqk[:])
        for sc in range(1, NSQ):
            nc.sync.dma_start(
                xT_sb[:, :, sc * SQ:(sc + 1) * SQ], xT[:, :, sc * SQ:(sc + 1) * SQ]
            )
        nc.sync.dma_start(wv_sb[:], wv[:])
        nc.sync.dma_start(wqk_sb[:, 1:3, :, :], wqk[:, 1:3, :, :])
        nc.sync.dma_start(bv_sb[:], bv[:])
        nc.sync.dma_start(wo01_sb[:], wo01[:])
        nc.sync.dma_start(wo2_sb[:], wo2[:])
        # ones columns next to each head's v block (softmax denominator trick)
        nc.gpsimd.memset(
            vp[:].rearrange("p s (h m) -> p s h m", m=128)[:, :, :, 64:128], 1.0
        )

        def qk_unit(h, sc):
            # one query-chunk of q/k projection for head h (+ bias)
            ps = ACC.tile([128, SQ], F32, tag="acc", name=f"qkps{h}_{sc}")
            for o in range(NDC):
                nc.tensor.matmul(
                    ps[:],
                    wqk_sb[:, h, o, :],
                    xT_sb[:, o, sc * SQ:(sc + 1) * SQ],
                    start=(o == 0),
                    stop=(o == NDC - 1),
                )
            nc.vector.tensor_tensor(
                qT[h][:, sc * SQ:(sc + 1) * SQ],
                ps[0:64, :],
                bqk_sb[0:64, h:h + 1].to_broadcast([64, SQ]),
                ADD,
            )
            # partition-shifted copy: psum rows 64..127 -> kT rows 0..63
            nc.vector.tensor_tensor(
                kT[h][:, sc * SQ:(sc + 1) * SQ],
                ps[64:128, :],
                bqk_sb[64:128, h:h + 1].to_broadcast([64, SQ]),
                ADD,
            )

        def v_unit(sc):
            # one key-chunk of v = xT.T @ [Wv_h0|Wv_h1|Wv_h2|pad] (+ bias)
            ps = ACC.tile([128, 256], F32, tag="acc", name=f"vps{sc}")
            for o in range(NDC):
                nc.tensor.matmul(
                    ps[:],
                    xT_sb[:, o, sc * 128:(sc + 1) * 128],
                    wv_sb[:, o, :],
                    start=(o == 0),
                    stop=(o == NDC - 1),
                )
            nc.vector.tensor_tensor(
                vp[:, sc, :].rearrange("p (h m) -> p h m", m=128)[:, :, 0:64],
                ps[:, 0:HPC * 64].rearrange("p (h m) -> p h m", m=64),
                bv_sb[:].rearrange("p (h m) -> p h m", m=64),
                ADD,
            )

        def proj_stage1(sc, ms, ctx01, store):
            # first half of the head-accumulation: ctx01.T @ Wo01 (ctx01 is
            # ready one block before ctx2, so this can weave into block h2)
            tiles = []
            for n0, nw in ((0, 512), (512, 256)):
                ops_t = ACC.tile([128, nw], F32, tag="acc", name=f"ops{sc}_{ms}_{n0}")
                nc.tensor.matmul(
                    ops_t[:],
                    ctx01[:, ms * 128:(ms + 1) * 128],
                    wo01_sb[:, n0:n0 + nw],
                    start=True,
                    stop=False,
                )
                tiles.append((n0, nw, ops_t))
            store[ms] = tiles

        def proj_stage2(sc, ms, ctx2, store):
            ot = W2.tile([128, D], F32, tag="out", name=f"ot{sc}_{ms}")
            for n0, nw, ops_t in store.pop(ms):
                nc.tensor.matmul(
                    ops_t[:],
                    ctx2[:, ms * 128:(ms + 1) * 128],
                    wo2_sb[:, n0:n0 + nw],
                    start=False,
                    stop=True,
                )
                nc.vector.tensor_copy(ot[:, n0:n0 + nw], ops_t[:])
            nc.sync.dma_start(
                out[(sc * 4 + ms) * 128:(sc * 4 + ms + 1) * 128, :], ot[:]
            )

        def proj_unit(sc, ms, ctx01, ctx2):
            store = {}
            proj_stage1(sc, ms, ctx01, store)
            proj_stage2(sc, ms, ctx2, store)

        filler = deque()
        stores = {}

        def attention_block(sc, h, ctx01, ctx2, pops_per_j=1, pop_stride=1):
            # probs stored flat [128, NSK*SQ]; exp runs 1024 wide over a
            # pair of key-chunk score tiles sharing one [128, 1024] psum.
            # probs@V matmuls are woven in with a one-pair lag; filler units
            # (other projections) are popped between pairs to keep the PE
            # stream busy while ACT chews on exps.
            probs = PR.tile([128, NSK * SQ], F16, tag="probs", name=f"pr{sc}_{h}")
            cps = ACC.tile([128, SQ], F32, tag="acc", name=f"cps{sc}_{h}")

            def probsv(mk):
                nc.tensor.matmul(
                    cps[:],
                    vp[:, mk, h * 128:(h + 1) * 128],
                    probs[:, mk * SQ:(mk + 1) * SQ],
                    start=(mk == 0),
                    stop=(mk == NSK - 1),
                )

            for j in range(NSK // 2):
                sps = SPS.tile([128, 2 * SQ], F32, tag="sps", name=f"sps{sc}_{h}_{j}")
                for half in range(2):
                    mk = 2 * j + half
                    nc.tensor.matmul(
                        sps[:, half * SQ:(half + 1) * SQ],
                        kT[h][:, mk * 128:(mk + 1) * 128],
                        qT[h][:, sc * SQ:(sc + 1) * SQ],
                        start=True,
                        stop=True,
                    )
                # probs = exp(scores / sqrt(DH)); no max-subtraction needed
                # (scores ~ N(0,1): exp can't overflow fp32/bf16)
                nc.scalar.activation(
                    probs[:, j * 2 * SQ:(j + 1) * 2 * SQ], sps[:], EXP,
                    scale=0.125,
                )
                if j % pop_stride == 0:
                    for _ in range(pops_per_j):
                        if filler:
                            filler.popleft()()
                if j > 0:
                    probsv(2 * (j - 1))
                    probsv(2 * j - 1)
            probsv(NSK - 2)
            probsv(NSK - 1)
            # rows 0..63: unnormalized ctxT; rows 64..127: denominators
            r = W2.tile([64, SQ], F32, tag="recip", name=f"r{sc}_{h}")
            nc.vector.reciprocal(r[:], cps[64:128, :])
            dst = ctx01[h * 64:(h + 1) * 64, :] if h < 2 else ctx2[:]
            nc.vector.tensor_tensor(dst, cps[0:64, :], r[:], MULT)

        # PE warmup: the HAM clock gate needs ~3-4us of sustained activity
        # to release full clock. The first real matmuls wait on DMA anyway,
        # so burn the wait on dummy matmuls over a resident zero tile — the
        # p-state ramp completes before real work starts.
        warm = P1.tile([64, 512], F32R, tag="warm")
        nc.vector.memset(warm[:].bitcast(F32), 0.0)
        wps = ACC.tile([128, 512], F32, tag="acc", name="warmps")
        for _ in range(10):
            nc.tensor.matmul(wps[:], warm[:, 0:128], warm[:], start=True, stop=True)
        # pre-load the ACT exp table set during the same dead time
        wact = P1.tile([64, 1], F16, tag="wact")
        nc.scalar.activation(wact[:], warm[:, 0:1].bitcast(F32), EXP, scale=0.125)

        # first two query-chunks of head-0 q/k run un-woven; scores pair j
        # of the first block needs kT columns only up to chunk (2j+1)//4, so
        # chunks 2-3 weave in as the block's leading fillers
        qk_unit(0, 0)

        ctxs = {}
        for sc in range(NSQ):
            ctxs[sc] = (
                W2.tile([128, SQ], F32R, tag="ctx01", name=f"c01_{sc}"),
                W2.tile([64, SQ], F32R, tag="ctx2", name=f"c2_{sc}"),
            )
            stores.setdefault(sc, {})
            for h in range(HPC):
                pops = 1
                if sc == 0 and h == 0:
                    # weave the tail of qk0, v-projection (2 chunks per pair,
                    # staying ahead of the lagged probs@V consumers) and
                    # head-1 q/k projection
                    filler.append(lambda: qk_unit(0, 1))
                    filler.append(lambda: qk_unit(0, 2))
                    filler.append(lambda: qk_unit(0, 3))
                    for i in range(NSK // 2):
                        filler.append(lambda i=i: v_unit(2 * i))
                        filler.append(lambda i=i: v_unit(2 * i + 1))
                        if i % 2 == 0:
                            filler.append(lambda i=i: qk_unit(1, i // 2))
                    pops = 3
                elif sc == 0 and h == 1:
                    for i in range(NSQ):
                        filler.append(lambda i=i: qk_unit(2, i))
                elif sc == NSQ - 1 and h == HPC - 1:
                    # start the last chunk's out-projection inside the last
                    # block (ctx01 is ready; only ctx2 accumulation waits)
                    filler.append(
                        lambda: proj_stage1(sc, 0, ctxs[sc][0], stores[sc])
                    )
                stride = 3 if sc > 0 else (3 if h == 1 else 1)
                attention_block(sc, h, *ctxs[sc], pops_per_j=pops,
                                pop_stride=stride)
            # out-projection of this chunk becomes filler for the next chunk,
            # split into its two accumulation stages so psum slots recycle
            for ms in range(SQ // 128):
                if sc == NSQ - 1 and ms == 0:
                    filler.append(
                        lambda sc=sc: proj_stage2(sc, 0, ctxs[sc][1], stores[sc])
                    )
                    continue
                filler.append(
                    lambda sc=sc, ms=ms: proj_stage1(sc, ms, ctxs[sc][0], stores[sc])
                )
                filler.append(
                    lambda sc=sc, ms=ms: proj_stage2(sc, ms, ctxs[sc][1], stores[sc])
                )
        while filler:
            filler.popleft()()


_CACHE = {}


def _get_module():
    if "nc" not in _CACHE:
        _CACHE["nc"] = _build_module()
    return _CACHE["nc"]


def make_in_maps(x, Wq, Wk, Wv, bq, bk, bv, Wo):
    f = np.float32
    in_maps = []
    for c in range(NCORES):
        b = c // CORES_PER_BATCH
        hh = [HPC * (c % CORES_PER_BATCH) + i for i in range(HPC)]
        # xT pre-tiled to [128, 6, 2048]: partition p, d-chunk o, seq s
        xt = x[b].T.reshape(NDC, 128, S).transpose(1, 0, 2)
        # wqk pre-tiled to [128, 3, 6, 128]
        wqk = np.stack(
            [np.concatenate([Wq[h], Wk[h]], axis=1) for h in hh]
        )  # [3, 768, 128]
        wqk = wqk.reshape(HPC, NDC, 128, 128).transpose(2, 0, 1, 3)
        # wv pre-tiled to [128, 6, 256]
        wv_stack = np.concatenate(
            [Wv[h] for h in hh] + [np.zeros((D, 64), f)], axis=1
        )  # [768, 256]
        wv_stack = wv_stack.reshape(NDC, 128, 256).transpose(1, 0, 2)
        in_maps.append({
            "xT": np.ascontiguousarray(xt).astype(f, copy=False),
            "wqk": np.ascontiguousarray(wqk).astype(f, copy=False),
            "wv": np.ascontiguousarray(wv_stack).astype(f, copy=False),
            "wo01": np.ascontiguousarray(Wo[hh[0] * DH:(hh[0] + 2) * DH, :]).astype(f, copy=False),
            "wo2": np.ascontiguousarray(Wo[hh[2] * DH:(hh[2] + 1) * DH, :]).astype(f, copy=False),
            "bqk": np.ascontiguousarray(
                np.stack([np.concatenate([bq[h], bk[h]]) for h in hh], axis=1)
            ).astype(f, copy=False),
            "bv": np.ascontiguousarray(
                np.tile(np.concatenate([bv[h] for h in hh]), (128, 1))
            ).astype(f, copy=False),
        })
    return in_maps


def gather(results, bo):
    out = np.empty((B, S, D), np.float32)
    for b in range(B):
        acc = results[b * CORES_PER_BATCH]["out"].astype(np.float32, copy=True)
        for c in range(b * CORES_PER_BATCH + 1, (b + 1) * CORES_PER_BATCH):
            acc += results[c]["out"]
        out[b] = acc + bo[None, :].astype(np.float32)
    return out


def kernel(x, Wq, Wk, Wv, bq, bk, bv, Wo, bo, c=0, **_unused):
    x, Wq, Wk, Wv, bq, bk, bv, Wo, bo = (
        np.asarray(a, np.float32) for a in (x, Wq, Wk, Wv, bq, bk, bv, Wo, bo)
    )
    nc = _get_module()
    in_maps = make_in_maps(x, Wq, Wk, Wv, bq, bk, bv, Wo)
    res = run_bass_kernel_spmd(nc, in_maps, list(range(NCORES)))
    return gather(res.results, bo)



# revision 3
# speedup vs baseline: 1.1224x; 1.1224x over previous
"""Multi-head attention kernel for Trainium2, sharded over 8 NeuronCores.

Sharding: data parallel over batch (B=2 -> 4 cores each) x tensor parallel
over heads (12 heads -> 3 heads per core). Each core computes QKV projections,
attention, and a partial output projection for its 3 heads; the per-head
partial output projections are summed on the host (the all-reduce of the
tensor-parallel hint, done during the gather step) and the output bias added.

Layout choices (per core):
  - All matmul operands are fp16 (x, W*, q, k, ctx, Wo) -- halves input DMA
    vs fp32 and keeps full matmul rate; scores/accumulations stay fp32 in
    PSUM.
  - x arrives pre-transposed and pre-tiled as xT [128, 6, 2048] so the
    contraction dim (d) sits on SBUF partitions for all QKV matmuls.
  - q and k are produced head-by-head directly in transposed form via a
    stacked weight [Wq_h | Wk_h]; scores are computed transposed
    (scoresT [s_k, s_q]) so the softmax probabilities feed the probs@V
    matmul with no transpose.
  - probs are written by the scalar engine as fp8e4m3 = exp(s/8 - 2); the
    -2 shift keeps the distribution inside e4m3's range. The shift cancels
    exactly in the softmax ratio.
  - probs@V runs in fp8 DoubleRow perf mode (2 contraction chunks per
    matmul at 2x rate). v rides as an fp8 hi+lo pair (lo = v - fp8(v),
    requantized) accumulated into the same PSUM group, so v keeps ~fp16
    accuracy while both DoubleRow operands are fp8.
  - softmax denominators ride for free: the v-hi operand carries a block of
    ones columns (zeros in v-lo), so rows 64..127 of the probs@V
    accumulation are the per-query sums of the same quantized probs --
    quantization partially cancels in the ratio.
  - the v bias never touches the device: ctx_h = sum(p v)/sum(p) + bv_h, so
    sum_h bv_h @ Wo_h folds into the output bias on the host.
  - emission order hand-weaves independent PE work (v/qk/out projections)
    into the ACT-paced scores->exp->probs@V pipeline; with probs@V halved
    by DoubleRow the kernel is ACT(exp)-bound, so the weave keeps the exp
    stream gap-free.
"""

from collections import deque

import numpy as np

import concourse.mybir as mybir
from concourse import bacc
from concourse.tile import TileContext
from concourse.bass_utils import run_bass_kernel_spmd

H, D, DH = 12, 768, 64
B, S = 2, 2048
NCORES = 8
CORES_PER_BATCH = 4
HPC = 3  # heads per core
SQ = 512  # query-chunk width
NSQ = S // SQ  # 4
NSK = S // 128  # 16 key chunks
NPR = NSK // 2  # 8 key-chunk pairs
NDC = D // 128  # 6 contraction chunks
VW = HPC * DH  # 192 v columns per core

F32 = mybir.dt.float32
F16 = mybir.dt.float16
F8 = mybir.dt.float8e4
ADD = mybir.AluOpType.add
SUB = mybir.AluOpType.subtract
MULT = mybir.AluOpType.mult
EXP = mybir.ActivationFunctionType.Exp
DRM = mybir.MatmulPerfMode.DoubleRow
EXP_SHIFT = -2.0  # probs = exp(s/8 - 2); cancels in the softmax ratio


def _build_module():
    nc = bacc.Bacc("TRN2", target_bir_lowering=False, debug=False, num_devices=NCORES)
    xT = nc.declare_dram_parameter("xT", [128, NDC, S], F16, isOutput=False)
    wqk = nc.declare_dram_parameter("wqk", [128, HPC, NDC, 128], F16, isOutput=False)
    wv = nc.declare_dram_parameter("wv", [128, NDC, VW], F16, isOutput=False)
    wo01 = nc.declare_dram_parameter("wo01", [128, D], F16, isOutput=False)
    wo2 = nc.declare_dram_parameter("wo2", [64, D], F16, isOutput=False)
    bqk = nc.declare_dram_parameter("bqk", [128, HPC], F32, isOutput=False)
    out = nc.declare_dram_parameter("out", [S, D], F32, isOutput=True)

    with TileContext(nc) as tc:
        _body(nc, tc, xT, wqk, wv, wo01, wo2, bqk, out)
    nc.compile()
    return nc


def _body(nc, tc, xT, wqk, wv, wo01, wo2, bqk, out):
    with (
        tc.tile_pool(name="persist", bufs=1) as P1,
        tc.tile_pool(name="work", bufs=4) as W2,
        tc.tile_pool(name="probs", bufs=2) as PR,
        # PSUM budget is 8 banks of [128, 512] fp32:
        #   ACC: one shared rotating pool for qk-proj, v-proj, ctx accum and
        #        out-proj tiles (4 banks)
        #   SPS: [128, 1024] score tiles, double-buffered (4 banks) -- pairs of
        #        key-chunks share one tile so exp runs 1024 wide
        tc.tile_pool(name="acc", bufs=4, space="PSUM") as ACC,
        tc.tile_pool(name="sps", bufs=2, space="PSUM") as SPS,
    ):
        xT_sb = P1.tile([128, NDC, S], F16, tag="xT")
        wqk_sb = P1.tile([128, HPC, NDC, 128], F16, tag="wqk")
        wv_sb = P1.tile([128, NDC, VW], F16, tag="wv")
        wo01_sb = P1.tile([128, D], F16, tag="wo01")
        wo2_sb = P1.tile([64, D], F16, tag="wo2")
        bqk_sb = P1.tile([128, HPC], F32, tag="bqk")
        ebias = P1.tile([128, 1], F32, tag="ebias")
        # q/k transposed per head (separate tiles: matmul operands must share
        # the SBUF base partition, so both live at partitions 0..63)
        qT = [
            P1.tile([64, S], F16, tag=f"qT{h}", name=f"qT{h}")
            for h in range(HPC)
        ]
        kT = [
            P1.tile([64, S], F16, tag=f"kT{h}", name=f"kT{h}")
            for h in range(HPC)
        ]
        # v hi/lo fp8 tiles: per chunk-pair c and parity j, per head a
        # [v (64) | ones (64)] block (ones in hi, zeros in lo)
        vph = P1.tile([128, NPR, 2, HPC * 128], F8, tag="vph")
        vpl = P1.tile([128, NPR, 2, HPC * 128], F8, tag="vpl")

        # DMAs in first-needed order. Each dma_start pays ~1.3us of serialized
        # HWDGE/DGE overhead plus 0.9us sem latency, so batch big -- except
        # the first qk unit's dependencies (wqk head 0, xT query-chunk 0
        # split per d-chunk), which are split fine so PE can start early.
        nc.sync.dma_start(wqk_sb[:, 0, :, :], wqk[:, 0, :, :])
        for o in range(NDC):
            nc.sync.dma_start(xT_sb[:, o, 0:SQ], xT[:, o, 0:SQ])
        nc.sync.dma_start(bqk_sb[:], bqk[:])
        for sc in range(1, NSQ):
            nc.sync.dma_start(
                xT_sb[:, :, sc * SQ:(sc + 1) * SQ], xT[:, :, sc * SQ:(sc + 1) * SQ]
            )
        nc.sync.dma_start(wv_sb[:], wv[:])
        nc.sync.dma_start(wqk_sb[:, 1:3, :, :], wqk[:, 1:3, :, :])
        nc.sync.dma_start(wo01_sb[:], wo01[:])
        nc.sync.dma_start(wo2_sb[:], wo2[:])
        nc.vector.memset(ebias[:], EXP_SHIFT)
        # ones columns next to each head's v-hi block (softmax denominator
        # trick); the v-lo ones-region must be zero (no double count)
        nc.gpsimd.memset(
            vph[:].rearrange("p c j (h m) -> p c j h m", m=128)[:, :, :, :, 64:128],
            1.0,
        )
        nc.gpsimd.memset(
            vpl[:].rearrange("p c j (h m) -> p c j h m", m=128)[:, :, :, :, 64:128],
            0.0,
        )

        def qk_unit(h, sc):
            # one query-chunk of q/k projection for head h (+ bias)
            ps = ACC.tile([128, SQ], F32, tag="acc", name=f"qkps{h}_{sc}")
            for o in range(NDC):
                nc.tensor.matmul(
                    ps[:],
                    wqk_sb[:, h, o, :],
                    xT_sb[:, o, sc * SQ:(sc + 1) * SQ],
                    start=(o == 0),
                    stop=(o == NDC - 1),
                )
            nc.vector.tensor_tensor(
                qT[h][:, sc * SQ:(sc + 1) * SQ],
                ps[0:64, :],
                bqk_sb[0:64, h:h + 1].to_broadcast([64, SQ]),
                ADD,
            )
            # partition-shifted copy: psum rows 64..127 -> kT rows 0..63
            nc.vector.tensor_tensor(
                kT[h][:, sc * SQ:(sc + 1) * SQ],
                ps[64:128, :],
                bqk_sb[64:128, h:h + 1].to_broadcast([64, SQ]),
                ADD,
            )

        def v_unit(mk):
            # one key-chunk of v = xT.T @ [Wv_h0|Wv_h1|Wv_h2], split into
            # fp8 hi + lo (no bias: bv folds into bo on the host)
            c, j = mk // 2, mk % 2
            ps = ACC.tile([128, VW], F32, tag="acc", name=f"vps{mk}")
            for o in range(NDC):
                nc.tensor.matmul(
                    ps[:],
                    xT_sb[:, o, mk * 128:(mk + 1) * 128],
                    wv_sb[:, o, :],
                    start=(o == 0),
                    stop=(o == NDC - 1),
                )
            hi = vph[:, c, j, :].rearrange("p (h m) -> p h m", m=128)[:, :, 0:64]
            lo = vpl[:, c, j, :].rearrange("p (h m) -> p h m", m=128)[:, :, 0:64]
            psv = ps[:].rearrange("p (h m) -> p h m", m=64)
            nc.vector.tensor_copy(hi, psv)
            nc.vector.tensor_tensor(lo, psv, hi, SUB)

        def proj_stage1(sc, ms, ctx01, store):
            # first half of the head-accumulation: ctx01.T @ Wo01 (ctx01 is
            # ready one block before ctx2, so this can weave into block h2)
            tiles = []
            for n0, nw in ((0, 512), (512, 256)):
                ops_t = ACC.tile([128, nw], F32, tag="acc", name=f"ops{sc}_{ms}_{n0}")
                nc.tensor.matmul(
                    ops_t[:],
                    ctx01[:, ms * 128:(ms + 1) * 128],
                    wo01_sb[:, n0:n0 + nw],
                    start=True,
                    stop=False,
                )
                tiles.append((n0, nw, ops_t))
            store[ms] = tiles

        def proj_stage2(sc, ms, ctx2, store):
            ot = W2.tile([128, D], F32, tag="out", name=f"ot{sc}_{ms}")
            for n0, nw, ops_t in store.pop(ms):
                nc.tensor.matmul(
                    ops_t[:],
                    ctx2[:, ms * 128:(ms + 1) * 128],
                    wo2_sb[:, n0:n0 + nw],
                    start=False,
                    stop=True,
                )
                nc.vector.tensor_copy(ot[:, n0:n0 + nw], ops_t[:])
            nc.sync.dma_start(
                out[(sc * 4 + ms) * 128:(sc * 4 + ms + 1) * 128, :], ot[:]
            )

        filler = deque()
        stores = {}

        def attention_block(sc, h, ctx01, ctx2, pops_per_j=1, pop_stride=1):
            # probs stored flat [128, NSK*SQ] fp8; exp runs 1024 wide over a
            # pair of key-chunk score tiles sharing one [128, 1024] psum.
            # probs@V DoubleRow matmuls (hi+lo per pair) are woven in with a
            # one-pair lag; filler units (other projections) are popped
            # between pairs to keep the PE stream busy while ACT chews exps.
            probs = PR.tile([128, NSK * SQ], F8, tag="probs", name=f"pr{sc}_{h}")
            cps = ACC.tile([128, SQ], F32, tag="acc", name=f"cps{sc}_{h}")

            def probsv(c):
                pr = probs[:, c * 2 * SQ:(c + 1) * 2 * SQ].rearrange(
                    "p (j n) -> p j n", j=2
                )
                nc.tensor.matmul(
                    cps[:],
                    vph[:, c, :, h * 128:(h + 1) * 128],
                    pr,
                    start=(c == 0),
                    stop=False,
                    perf_mode=DRM,
                )
                nc.tensor.matmul(
                    cps[:],
                    vpl[:, c, :, h * 128:(h + 1) * 128],
                    pr,
                    start=False,
                    stop=(c == NPR - 1),
                    perf_mode=DRM,
                )

            for j in range(NPR):
                sps = SPS.tile([128, 2 * SQ], F32, tag="sps", name=f"sps{sc}_{h}_{j}")
                for half in range(2):
                    mk = 2 * j + half
                    nc.tensor.matmul(
                        sps[:, half * SQ:(half + 1) * SQ],
                        kT[h][:, mk * 128:(mk + 1) * 128],
                        qT[h][:, sc * SQ:(sc + 1) * SQ],
                        start=True,
                        stop=True,
                    )
                # probs = exp(scores/sqrt(DH) - 2) in fp8e4m3; the -2 shift
                # keeps the top inside e4m3 range and cancels in the ratio
                nc.scalar.activation(
                    probs[:, j * 2 * SQ:(j + 1) * 2 * SQ], sps[:], EXP,
                    scale=0.125, bias=ebias[:],
                )
                if j % pop_stride == 0:
                    for _ in range(pops_per_j):
                        if filler:
                            filler.popleft()()
                if j > 0:
                    probsv(j - 1)
            probsv(NPR - 1)
            # rows 0..63: unnormalized ctxT; rows 64..127: denominators
            r = W2.tile([64, SQ], F32, tag="recip", name=f"r{sc}_{h}")
            nc.vector.reciprocal(r[:], cps[64:128, :])
            dst = ctx01[h * 64:(h + 1) * 64, :] if h < 2 else ctx2[:]
            nc.vector.tensor_tensor(dst, cps[0:64, :], r[:], MULT)

        # PE warmup: the HAM clock gate needs ~3-4us of sustained activity
        # to release full clock. The first real matmuls wait on DMA anyway,
        # so burn the wait on dummy matmuls over a resident zero tile -- the
        # p-state ramp completes before real work starts.
        warm = P1.tile([64, 512], F16, tag="warm")
        nc.vector.memset(warm[:].bitcast(F32), 0.0)
        wps = ACC.tile([128, 512], F32, tag="acc", name="warmps")
        for _ in range(10):
            nc.tensor.matmul(wps[:], warm[:, 0:128], warm[:], start=True, stop=True)
        # pre-load the ACT exp table set during the same dead time
        wact = P1.tile([64, 1], F16, tag="wact")
        nc.scalar.activation(wact[:], warm[:, 0:2].bitcast(F32), EXP, scale=0.125)

        # first two query-chunks of head-0 q/k run un-woven; scores pair j
        # of the first block needs kT columns only up to chunk (2j+1)//4, so
        # chunks 2-3 weave in as the block's leading fillers
        qk_unit(0, 0)

        ctxs = {}
        for sc in range(NSQ):
            ctxs[sc] = (
                W2.tile([128, SQ], F16, tag="ctx01", name=f"c01_{sc}"),
                W2.tile([64, SQ], F16, tag="ctx2", name=f"c2_{sc}"),
            )
            stores.setdefault(sc, {})
            for h in range(HPC):
                pops = 1
                if sc == 0 and h == 0:
                    # weave the tail of qk0, v-projection (2 chunks per pair,
                    # staying ahead of the lagged probs@V consumers) and
                    # head-1 q/k projection
                    filler.append(lambda: qk_unit(0, 1))
                    filler.append(lambda: qk_unit(0, 2))
                    filler.append(lambda: qk_unit(0, 3))
                    for i in range(NPR):
                        filler.append(lambda i=i: v_unit(2 * i))
                        filler.append(lambda i=i: v_unit(2 * i + 1))
                        if i % 2 == 0:
                            filler.append(lambda i=i: qk_unit(1, i // 2))
                    pops = 3
                elif sc == 0 and h == 1:
                    for i in range(NSQ):
                        filler.append(lambda i=i: qk_unit(2, i))
                elif sc == NSQ - 1 and h == HPC - 1:
                    # start the last chunk's out-projection inside the last
                    # block (ctx01 is ready; only ctx2 accumulation waits)
                    filler.append(
                        lambda: proj_stage1(sc, 0, ctxs[sc][0], stores[sc])
                    )
                stride = 3 if sc > 0 else (3 if h == 1 else 1)
                attention_block(sc, h, *ctxs[sc], pops_per_j=pops,
                                pop_stride=stride)
            # out-projection of this chunk becomes filler for the next chunk,
            # split into its two accumulation stages so psum slots recycle
            for ms in range(SQ // 128):
                if sc == NSQ - 1 and ms == 0:
                    filler.append(
                        lambda sc=sc: proj_stage2(sc, 0, ctxs[sc][1], stores[sc])
                    )
                    continue
                filler.append(
                    lambda sc=sc, ms=ms: proj_stage1(sc, ms, ctxs[sc][0], stores[sc])
                )
                filler.append(
                    lambda sc=sc, ms=ms: proj_stage2(sc, ms, ctxs[sc][1], stores[sc])
                )
        while filler:
            filler.popleft()()


_CACHE = {}


def _get_module():
    if "nc" not in _CACHE:
        _CACHE["nc"] = _build_module()
    return _CACHE["nc"]


def make_in_maps(x, Wq, Wk, Wv, bq, bk, bv, Wo):
    f16 = np.float16
    in_maps = []
    for c in range(NCORES):
        b = c // CORES_PER_BATCH
        hh = [HPC * (c % CORES_PER_BATCH) + i for i in range(HPC)]
        # xT pre-tiled to [128, 6, 2048]: partition p, d-chunk o, seq s
        xt = x[b].T.reshape(NDC, 128, S).transpose(1, 0, 2)
        # wqk pre-tiled to [128, 3, 6, 128]
        wqk = np.stack(
            [np.concatenate([Wq[h], Wk[h]], axis=1) for h in hh]
        )  # [3, 768, 128]
        wqk = wqk.reshape(HPC, NDC, 128, 128).transpose(2, 0, 1, 3)
        # wv pre-tiled to [128, 6, 192]
        wv_stack = np.concatenate([Wv[h] for h in hh], axis=1)  # [768, 192]
        wv_stack = wv_stack.reshape(NDC, 128, VW).transpose(1, 0, 2)
        in_maps.append({
            "xT": np.ascontiguousarray(xt).astype(f16),
            "wqk": np.ascontiguousarray(wqk).astype(f16),
            "wv": np.ascontiguousarray(wv_stack).astype(f16),
            "wo01": np.ascontiguousarray(
                Wo[hh[0] * DH:(hh[0] + 2) * DH, :]
            ).astype(f16),
            "wo2": np.ascontiguousarray(
                Wo[hh[2] * DH:(hh[2] + 1) * DH, :]
            ).astype(f16),
            "bqk": np.ascontiguousarray(
                np.stack([np.concatenate([bq[h], bk[h]]) for h in hh], axis=1)
            ).astype(np.float32),
        })
    return in_maps


def gather(results, bv, Wo, bo):
    # ctx_h = softmax(scores) @ v_nobias + bv_h, so the bv contribution to
    # the output is a constant row: sum_h bv_h @ Wo_h, folded into bo here.
    bo_eff = bo.astype(np.float64) + bv.reshape(-1).astype(np.float64) @ Wo.astype(
        np.float64
    )
    out = np.empty((B, S, D), np.float32)
    for b in range(B):
        acc = results[b * CORES_PER_BATCH]["out"].astype(np.float64, copy=True)
        for c in range(b * CORES_PER_BATCH + 1, (b + 1) * CORES_PER_BATCH):
            acc += results[c]["out"]
        out[b] = (acc + bo_eff[None, :]).astype(np.float32)
    return out


def kernel(x, Wq, Wk, Wv, bq, bk, bv, Wo, bo, c=0, **_unused):
    x, Wq, Wk, Wv, bq, bk, bv, Wo, bo = (
        np.asarray(a, np.float32) for a in (x, Wq, Wk, Wv, bq, bk, bv, Wo, bo)
    )
    nc = _get_module()
    in_maps = make_in_maps(x, Wq, Wk, Wv, bq, bk, bv, Wo)
    res = run_bass_kernel_spmd(nc, in_maps, list(range(NCORES)))
    return gather(res.results, bv, Wo, bo)


# revision 5
# speedup vs baseline: 1.2056x; 1.0741x over previous
"""Multi-head attention kernel for Trainium2, sharded over 8 NeuronCores.

Sharding: data parallel over batch (B=2 -> 4 cores each) x tensor parallel
over heads (12 heads -> 3 heads per core). Each core computes QKV projections,
attention, and a partial output projection for its 3 heads; the per-head
partial output projections are summed on the host (the all-reduce of the
tensor-parallel hint, done during the gather step) and the output bias added.

Design (per core):
  - All matmul operands are fp16 (x, W*, q, k, ctx, Wo) -- halves input DMA
    vs fp32 at full matmul rate; accumulations stay fp32 in PSUM. Partial
    outputs return as fp16 (summed in fp32 on the host).
  - x arrives pre-transposed and pre-tiled as xT [128, 6, 2048] so the
    contraction dim (d) sits on SBUF partitions for all QKV matmuls.
  - q and k are produced head-by-head directly in transposed form via a
    stacked weight [Wq_h | Wk_h]; scores are computed transposed
    (scoresT [s_k, s_q]) so the softmax probabilities feed the probs@V
    matmul with no transpose.
  - probs are written by the scalar engine as fp8e4m3 = exp(s/8 - 2); the
    -2 shift keeps the distribution inside e4m3's range and cancels exactly
    in the softmax ratio.
  - probs@V runs in fp8 DoubleRow perf mode (two 128-deep contraction
    chunks per matmul at 2x rate). v rides as an fp8 hi+lo pair
    (lo = fp8(v - fp8(v))) accumulated into the same PSUM group, so v keeps
    ~fp16 accuracy while both DoubleRow operands are fp8.
  - softmax denominators ride for free: the v-hi operand carries a block of
    ones columns (zeros in v-lo), so rows 64..127 of the probs@V
    accumulation are the per-query sums of the same quantized probs --
    quantization partially cancels in the ratio.
  - the v bias never touches the device: ctx_h = sum(p v)/sum(p) + bv_h, so
    sum_h bv_h @ Wo_h folds into the output bias on the host.
  - with probs@V halved by DoubleRow the kernel is ACT(exp)-bound, so the
    emission scheduler paces one [128,1024] exp per ~1038ns slot and packs
    all other PE work (qk/v/out projections, lagged DoubleRow matmuls) into
    the leftover budget of each slot, deadline-ordered. Blocks run
    head-major so each phase only needs its own head's q/k projections.
"""

import heapq
from itertools import count

import numpy as np

import concourse.mybir as mybir
from concourse import bacc
from concourse.tile import TileContext
from concourse.bass_utils import run_bass_kernel_spmd

H, D, DH = 12, 768, 64
B, S = 2, 2048
NCORES = 8
CORES_PER_BATCH = 4
HPC = 3  # heads per core
SQ = 512  # query-chunk width
NSQ = S // SQ  # 4
NSK = S // 128  # 16 key chunks
NPR = NSK // 2  # 8 key-chunk pairs
NDC = D // 128  # 6 contraction chunks
VW = HPC * DH  # 192 v columns per core
NBLK = HPC * NSQ  # 12 attention blocks, head-major: b = h*NSQ + sc

F32 = mybir.dt.float32
F16 = mybir.dt.float16
F8 = mybir.dt.float8e4
ADD = mybir.AluOpType.add
SUB = mybir.AluOpType.subtract
MULT = mybir.AluOpType.mult
EXP = mybir.ActivationFunctionType.Exp
DRM = mybir.MatmulPerfMode.DoubleRow
EXP_SHIFT = -2.0  # probs = exp(s/8 - 2); cancels in the softmax ratio

# cost-model pacing constants (ns)
SLOT = 1038.0  # one [128,1024] exp on ACT
C_SCORE = 426.0  # two [128,512] fp16 score matmuls
C_QKMM = 213.0
C_VUNIT = 480.0
C_DR = 214.0  # one hi+lo DoubleRow pair
C_PROJ = 640.0


def _build_module():
    nc = bacc.Bacc("TRN2", target_bir_lowering=False, debug=False, num_devices=NCORES)
    xT = nc.declare_dram_parameter("xT", [128, NDC, S], F16, isOutput=False)
    wqk = nc.declare_dram_parameter("wqk", [128, HPC, NDC, 128], F16, isOutput=False)
    wv = nc.declare_dram_parameter("wv", [128, NDC, VW], F16, isOutput=False)
    wo01 = nc.declare_dram_parameter("wo01", [128, D], F16, isOutput=False)
    wo2 = nc.declare_dram_parameter("wo2", [64, D], F16, isOutput=False)
    bqk = nc.declare_dram_parameter("bqk", [128, HPC], F32, isOutput=False)
    out = nc.declare_dram_parameter("out", [S, D], F16, isOutput=True)

    with TileContext(nc) as tc:
        _body(nc, tc, xT, wqk, wv, wo01, wo2, bqk, out)
    nc.compile()
    return nc


def _body(nc, tc, xT, wqk, wv, wo01, wo2, bqk, out):
    with (
        tc.tile_pool(name="persist", bufs=1) as P1,
        tc.tile_pool(name="work", bufs=4) as W2,
        tc.tile_pool(name="probs", bufs=3) as PR,
        # PSUM budget is 8 banks of [128, 512] fp32:
        #   ACC: one shared rotating pool for qk-proj, v-proj, ctx accum and
        #        out-proj tiles (4 banks)
        #   SPS: [128, 1024] score tiles, double-buffered (4 banks) -- pairs
        #        of key-chunks share one tile so exp runs 1024 wide
        tc.tile_pool(name="acc", bufs=4, space="PSUM") as ACC,
        tc.tile_pool(name="sps", bufs=2, space="PSUM") as SPS,
    ):
        xT_sb = P1.tile([128, NDC, S], F16, tag="xT")
        wqk_sb = P1.tile([128, HPC, NDC, 128], F16, tag="wqk")
        wv_sb = P1.tile([128, NDC, VW], F16, tag="wv")
        wo01_sb = P1.tile([128, D], F16, tag="wo01")
        wo2_sb = P1.tile([64, D], F16, tag="wo2")
        bqk_sb = P1.tile([128, HPC], F32, tag="bqk")
        ebias = P1.tile([128, 1], F32, tag="ebias")
        qT = [P1.tile([64, S], F16, tag=f"qT{h}", name=f"qT{h}") for h in range(HPC)]
        kT = [P1.tile([64, S], F16, tag=f"kT{h}", name=f"kT{h}") for h in range(HPC)]
        # v hi/lo fp8 tiles: per chunk-pair c and parity j, per head a
        # [v (64) | ones (64)] block (ones in hi, zeros in lo)
        vph = P1.tile([128, NPR, 2, HPC * 128], F8, tag="vph")
        vpl = P1.tile([128, NPR, 2, HPC * 128], F8, tag="vpl")

        # DMAs in first-needed order. Each dma_start pays ~1.3us of
        # serialized HWDGE/DGE overhead plus 0.9us sem latency, so batch
        # big -- except the first qk unit's dependencies (wqk head 0, xT
        # query-chunk 0 split per d-chunk) so PE can start early.
        nc.sync.dma_start(wqk_sb[:, 0, :, :], wqk[:, 0, :, :])
        for o in range(NDC):
            nc.sync.dma_start(xT_sb[:, o, 0:SQ], xT[:, o, 0:SQ])
        nc.sync.dma_start(bqk_sb[:], bqk[:])
        for sc in range(1, NSQ):
            nc.sync.dma_start(
                xT_sb[:, :, sc * SQ:(sc + 1) * SQ], xT[:, :, sc * SQ:(sc + 1) * SQ]
            )
        nc.sync.dma_start(wv_sb[:], wv[:])
        nc.sync.dma_start(wqk_sb[:, 1:3, :, :], wqk[:, 1:3, :, :])
        nc.sync.dma_start(wo01_sb[:], wo01[:])
        nc.sync.dma_start(wo2_sb[:], wo2[:])
        nc.vector.memset(ebias[:], EXP_SHIFT)
        # ones columns next to each head's v-hi block (softmax denominator
        # trick); the v-lo ones-region must be zero (no double count)
        nc.gpsimd.memset(
            vph[:].rearrange("p c j (h m) -> p c j h m", m=128)[:, :, :, :, 64:128],
            1.0,
        )
        nc.gpsimd.memset(
            vpl[:].rearrange("p c j (h m) -> p c j h m", m=128)[:, :, :, :, 64:128],
            0.0,
        )

        # ---- emission scheduler state -------------------------------------
        emitted = set()  # readiness flags
        heap = []  # (deadline, seq, cost, flags_needed, fn)
        pending = []  # items whose flags aren't satisfied yet
        seq = count()
        credit = [0.0]
        slot = [0]

        def add(deadline, cost, fn, needs=()):
            item = (deadline, next(seq), cost, tuple(needs), fn)
            if all(f in emitted for f in item[3]):
                heapq.heappush(heap, item)
            else:
                pending.append(item)

        def refresh():
            still = []
            for item in pending:
                if all(f in emitted for f in item[3]):
                    heapq.heappush(heap, item)
                else:
                    still.append(item)
            pending[:] = still

        def pump(force_overdue=True):
            # emit overdue items regardless of budget, then spend credit
            while heap:
                deadline, _, cost, _, fn = heap[0]
                if deadline <= slot[0] and force_overdue:
                    pass
                elif credit[0] >= cost:
                    pass
                else:
                    break
                heapq.heappop(heap)
                fn()
                credit[0] -= cost
                refresh()

        # ---- work units ---------------------------------------------------
        qk_ps = {}

        def qk_mm(h, q, o):
            if o == 0:
                qk_ps[h, q] = ACC.tile([128, SQ], F32, tag="acc", name=f"qkps{h}_{q}")
            ps = qk_ps[h, q]
            nc.tensor.matmul(
                ps[:],
                wqk_sb[:, h, o, :],
                xT_sb[:, o, q * SQ:(q + 1) * SQ],
                start=(o == 0),
                stop=(o == NDC - 1),
            )
            if o == NDC - 1:
                nc.vector.tensor_tensor(
                    qT[h][:, q * SQ:(q + 1) * SQ],
                    ps[0:64, :],
                    bqk_sb[0:64, h:h + 1].to_broadcast([64, SQ]),
                    ADD,
                )
                # partition-shifted copy: psum rows 64..127 -> kT rows 0..63
                nc.vector.tensor_tensor(
                    kT[h][:, q * SQ:(q + 1) * SQ],
                    ps[64:128, :],
                    bqk_sb[64:128, h:h + 1].to_broadcast([64, SQ]),
                    ADD,
                )
                emitted.add(f"qk{h}_{q}")

        def v_unit(mk):
            # one key-chunk of v = xT.T @ [Wv_h0|Wv_h1|Wv_h2], split into
            # fp8 hi + lo (no bias: bv folds into bo on the host)
            c, j = mk // 2, mk % 2
            ps = ACC.tile([128, VW], F32, tag="acc", name=f"vps{mk}")
            for o in range(NDC):
                nc.tensor.matmul(
                    ps[:],
                    xT_sb[:, o, mk * 128:(mk + 1) * 128],
                    wv_sb[:, o, :],
                    start=(o == 0),
                    stop=(o == NDC - 1),
                )
            hi = vph[:, c, j, :].rearrange("p (h m) -> p h m", m=128)[:, :, 0:64]
            lo = vpl[:, c, j, :].rearrange("p (h m) -> p h m", m=128)[:, :, 0:64]
            psv = ps[:].rearrange("p (h m) -> p h m", m=64)
            nc.vector.tensor_copy(hi, psv)
            nc.vector.tensor_tensor(lo, psv, hi, SUB)
            emitted.add(f"v{mk}")

        probs_t = {}
        cps_t = {}
        ctxs = [
            (
                W2.tile([128, SQ], F16, tag="ctx01", name=f"c01_{sc}"),
                W2.tile([64, SQ], F16, tag="ctx2", name=f"c2_{sc}"),
            )
            for sc in range(NSQ)
        ]

        def dr_pair(b, c):
            h, sc = b // NSQ, b % NSQ
            if c == 0:
                cps_t[b] = ACC.tile([128, SQ], F32, tag="acc", name=f"cps{b}")
            cps = cps_t[b]
            pr = probs_t[b][:, c * 2 * SQ:(c + 1) * 2 * SQ].rearrange(
                "p (j n) -> p j n", j=2
            )
            nc.tensor.matmul(
                cps[:], vph[:, c, :, h * 128:(h + 1) * 128], pr,
                start=(c == 0), stop=False, perf_mode=DRM,
            )
            nc.tensor.matmul(
                cps[:], vpl[:, c, :, h * 128:(h + 1) * 128], pr,
                start=False, stop=(c == NPR - 1), perf_mode=DRM,
            )
            emitted.add(f"dr{b}_{c}")
            if c == NPR - 1:
                finish_block(b)

        def finish_block(b):
            # rows 0..63: unnormalized ctxT; rows 64..127: denominators
            h, sc = b // NSQ, b % NSQ
            cps = cps_t.pop(b)
            ctx01, ctx2 = ctxs[sc]
            last = b == NBLK - 1
            pieces = 4 if last else 1
            w = SQ // pieces
            for i in range(pieces):
                r = W2.tile([64, w], F32, tag="recip", name=f"r{b}_{i}")
                nc.vector.reciprocal(r[:], cps[64:128, i * w:(i + 1) * w])
                dst = ctx01[h * 64:(h + 1) * 64, :] if h < 2 else ctx2[:]
                nc.vector.tensor_tensor(
                    dst[:, i * w:(i + 1) * w], cps[0:64, i * w:(i + 1) * w],
                    r[:], MULT,
                )
                for ms in range(i * w // 128, (i + 1) * w // 128):
                    emitted.add(f"ctx{b}_{ms}")
            emitted.add(f"blk{b}")

        def proj_unit(sc, ms):
            # out[sc,ms] = ctx01.T @ Wo01 + ctx2.T @ Wo2, copied out as fp16
            ctx01, ctx2 = ctxs[sc]
            ot = W2.tile([128, D], F16, tag="out", name=f"ot{sc}_{ms}")
            for n0, nw in ((0, 512), (512, 256)):
                ops_t = ACC.tile([128, nw], F32, tag="acc", name=f"ops{sc}_{ms}_{n0}")
                nc.tensor.matmul(
                    ops_t[:], ctx01[:, ms * 128:(ms + 1) * 128],
                    wo01_sb[:, n0:n0 + nw], start=True, stop=False,
                )
                nc.tensor.matmul(
                    ops_t[:], ctx2[:, ms * 128:(ms + 1) * 128],
                    wo2_sb[:, n0:n0 + nw], start=False, stop=True,
                )
                nc.vector.tensor_copy(ot[:, n0:n0 + nw], ops_t[:])
            nc.sync.dma_start(
                out[(sc * 4 + ms) * 128:(sc * 4 + ms + 1) * 128, :], ot[:]
            )

        # ---- static work list ---------------------------------------------
        # qk unit (h, q): six matmuls; needed (kT side) by pair 2q of block
        # (h, 0) at slot h*32 + 2q, minus slack for the DVE bias-add.
        for h in range(HPC):
            for q in range(NSQ):
                d = h * 32 + 2 * q - 2 - (1 if q == 0 else 0)
                for o in range(NDC):
                    add(d, C_QKMM, lambda h=h, q=q, o=o: qk_mm(h, q, o))
        # v units: needed by the lagged DoubleRow of block 0 onward
        for mk in range(NSK):
            add(6 + mk, C_VUNIT, lambda mk=mk: v_unit(mk))
        # DoubleRow probs@V pairs: blocks 0..3 (head-0 phase) may lag up to
        # the probs-buffer deadline (3 buffers); later blocks pair-lag so
        # ctx/proj complete in-phase.
        for b in range(NBLK):
            for c in range(NPR):
                d = 8 * (b + 3) - 1 if b < NSQ else 8 * b + c + 2
                add(d, C_DR, lambda b=b, c=c: dr_pair(b, c),
                    needs=(f"exp{b}_{c}", f"v{2 * c}", f"v{2 * c + 1}"))
        # out projections: after the h2 block of sc completes ctx2 (and
        # ctx01 long before). sc=3 lands in the tail, per-ms pipelined.
        for sc in range(NSQ):
            b2 = 2 * NSQ + sc
            for ms in range(4):
                add(8 * (b2 + 1) + 2 * ms, C_PROJ,
                    lambda sc=sc, ms=ms: proj_unit(sc, ms),
                    needs=(f"blk{NSQ + sc}", f"ctx{b2}_{ms}"))

        # PE warmup: the cost model's p-state ramp needs ~3us of sustained
        # matmul activity for full clock; the first real matmuls wait on DMA
        # anyway, so burn the wait on dummy matmuls over a resident tile.
        warm = P1.tile([64, 512], F16, tag="warm")
        nc.vector.memset(warm[:].bitcast(F32), 0.0)
        wps = ACC.tile([128, 512], F32, tag="acc", name="warmps")
        for _ in range(10):
            nc.tensor.matmul(wps[:], warm[:, 0:128], warm[:], start=True, stop=True)
        # pre-load the ACT exp table set during the same dead time
        wact = P1.tile([64, 1], F16, tag="wact")
        nc.scalar.activation(wact[:], warm[:, 0:2].bitcast(F32), EXP, scale=0.125)

        # ---- slot loop: one exp per slot, budget-paced fillers ------------
        for b in range(NBLK):
            h, sc = b // NSQ, b % NSQ
            probs_t[b] = PR.tile([128, NSK * SQ], F8, tag="probs", name=f"pr{b}")
            for j in range(NPR):
                pump()  # overdue first (qk deps for these scores)
                sps = SPS.tile([128, 2 * SQ], F32, tag="sps", name=f"sps{b}_{j}")
                for half in range(2):
                    mk = 2 * j + half
                    nc.tensor.matmul(
                        sps[:, half * SQ:(half + 1) * SQ],
                        kT[h][:, mk * 128:(mk + 1) * 128],
                        qT[h][:, sc * SQ:(sc + 1) * SQ],
                        start=True,
                        stop=True,
                    )
                nc.scalar.activation(
                    probs_t[b][:, j * 2 * SQ:(j + 1) * 2 * SQ], sps[:], EXP,
                    scale=0.125, bias=ebias[:],
                )
                emitted.add(f"exp{b}_{j}")
                refresh()
                credit[0] = min(credit[0] + SLOT - C_SCORE, 4 * SLOT)
                pump()
                slot[0] += 1
        # tail: drain everything left (last block's DR pairs, ctx, proj sc=3)
        credit[0] = 1e9
        while heap or pending:
            n0 = len(heap) + len(pending)
            pump()
            if len(heap) + len(pending) == n0:
                raise RuntimeError(
                    f"scheduler deadlock: {len(heap)} heap / {len(pending)} pending"
                )


_CACHE = {}


def _get_module():
    if "nc" not in _CACHE:
        _CACHE["nc"] = _build_module()
    return _CACHE["nc"]


def make_in_maps(x, Wq, Wk, Wv, bq, bk, bv, Wo):
    f16 = np.float16
    in_maps = []
    for c in range(NCORES):
        b = c // CORES_PER_BATCH
        hh = [HPC * (c % CORES_PER_BATCH) + i for i in range(HPC)]
        # xT pre-tiled to [128, 6, 2048]: partition p, d-chunk o, seq s
        xt = x[b].T.reshape(NDC, 128, S).transpose(1, 0, 2)
        # wqk pre-tiled to [128, 3, 6, 128]
        wqk = np.stack(
            [np.concatenate([Wq[h], Wk[h]], axis=1) for h in hh]
        )  # [3, 768, 128]
        wqk = wqk.reshape(HPC, NDC, 128, 128).transpose(2, 0, 1, 3)
        # wv pre-tiled to [128, 6, 192]
        wv_stack = np.concatenate([Wv[h] for h in hh], axis=1)  # [768, 192]
        wv_stack = wv_stack.reshape(NDC, 128, VW).transpose(1, 0, 2)
        in_maps.append({
            "xT": np.ascontiguousarray(xt).astype(f16),
            "wqk": np.ascontiguousarray(wqk).astype(f16),
            "wv": np.ascontiguousarray(wv_stack).astype(f16),
            "wo01": np.ascontiguousarray(
                Wo[hh[0] * DH:(hh[0] + 2) * DH, :]
            ).astype(f16),
            "wo2": np.ascontiguousarray(
                Wo[hh[2] * DH:(hh[2] + 1) * DH, :]
            ).astype(f16),
            "bqk": np.ascontiguousarray(
                np.stack([np.concatenate([bq[h], bk[h]]) for h in hh], axis=1)
            ).astype(np.float32),
        })
    return in_maps


def gather(results, bv, Wo, bo):
    # ctx_h = softmax(scores) @ v_nobias + bv_h, so the bv contribution to
    # the output is a constant row: sum_h bv_h @ Wo_h, folded into bo here.
    bo_eff = bo.astype(np.float64) + bv.reshape(-1).astype(np.float64) @ Wo.astype(
        np.float64
    )
    out = np.empty((B, S, D), np.float32)
    for b in range(B):
        acc = results[b * CORES_PER_BATCH]["out"].astype(np.float64, copy=True)
        for c in range(b * CORES_PER_BATCH + 1, (b + 1) * CORES_PER_BATCH):
            acc += results[c]["out"].astype(np.float64)
        out[b] = (acc + bo_eff[None, :]).astype(np.float32)
    return out


def kernel(x, Wq, Wk, Wv, bq, bk, bv, Wo, bo, c=0, **_unused):
    x, Wq, Wk, Wv, bq, bk, bv, Wo, bo = (
        np.asarray(a, np.float32) for a in (x, Wq, Wk, Wv, bq, bk, bv, Wo, bo)
    )
    nc = _get_module()
    in_maps = make_in_maps(x, Wq, Wk, Wv, bq, bk, bv, Wo)
    res = run_bass_kernel_spmd(nc, in_maps, list(range(NCORES)))
    return gather(res.results, bv, Wo, bo)


# revision 9
# speedup vs baseline: 1.2407x; 1.0291x over previous
"""Multi-head attention kernel for Trainium2, sharded over 8 NeuronCores.

Sharding: data parallel over batch (B=2 -> 4 cores each) x tensor parallel
over heads (12 heads -> 3 heads per core). Each core computes QKV projections,
attention, and a partial output projection for its 3 heads; the per-head
partial output projections are summed on the host (the all-reduce of the
tensor-parallel hint, done during the gather step) and the output bias added.

Design (per core):
  - All matmul operands are fp16 (x, W*, q, k, ctx, Wo) -- halves input DMA
    vs fp32 at full matmul rate; accumulations stay fp32 in PSUM. Partial
    outputs return as fp16 (summed in fp32 on the host).
  - x arrives pre-transposed and pre-tiled as xT [128, 6, 2048] so the
    contraction dim (d) sits on SBUF partitions for all QKV matmuls.
  - q and k are produced head-by-head directly in transposed form via a
    stacked weight [Wq_h | Wk_h]; scores are computed transposed
    (scoresT [s_k, s_q]) so the softmax probabilities feed the probs@V
    matmul with no transpose.
  - probs are written by the scalar engine as fp8e4m3 = exp(s/8 - 2); the
    -2 shift keeps the distribution inside e4m3's range and cancels exactly
    in the softmax ratio.
  - probs@V runs in fp8 DoubleRow perf mode (two 128-deep contraction
    chunks per matmul at 2x rate). v rides as an fp8 hi+lo pair
    (lo = fp8(v - fp8(v))) accumulated into the same PSUM group, so v keeps
    ~fp16 accuracy while both DoubleRow operands are fp8.
  - softmax denominators ride for free: the v-hi operand carries a block of
    ones columns (zeros in v-lo), so rows 64..127 of the probs@V
    accumulation are the per-query sums of the same quantized probs --
    quantization partially cancels in the ratio.
  - the v bias never touches the device: ctx_h = sum(p v)/sum(p) + bv_h, so
    sum_h bv_h @ Wo_h folds into the output bias on the host.
  - with probs@V halved by DoubleRow the kernel is ACT(exp)-bound, so the
    emission scheduler paces one [128,1024] exp per ~1038ns slot and packs
    all other PE work (qk/v/out projections, lagged DoubleRow matmuls) into
    the leftover budget of each slot, deadline-ordered. Blocks run
    head-major so each phase only needs its own head's q/k projections.
"""

import heapq
from itertools import count

import numpy as np

import concourse.mybir as mybir
from concourse import bacc
from concourse.tile import TileContext
from concourse.bass_utils import run_bass_kernel_spmd

H, D, DH = 12, 768, 64
B, S = 2, 2048
NCORES = 8
CORES_PER_BATCH = 4
HPC = 3  # heads per core
SQ = 512  # query-chunk width
NSQ = S // SQ  # 4
NSK = S // 128  # 16 key chunks
NPR = NSK // 2  # 8 key-chunk pairs
NDC = D // 128  # 6 contraction chunks
VW = HPC * DH  # 192 v columns per core
NBLK = HPC * NSQ  # 12 attention blocks, head-major: b = h*NSQ + sc

F32 = mybir.dt.float32
F16 = mybir.dt.float16
F8 = mybir.dt.float8e4
ADD = mybir.AluOpType.add
SUB = mybir.AluOpType.subtract
MULT = mybir.AluOpType.mult
EXP = mybir.ActivationFunctionType.Exp
DRM = mybir.MatmulPerfMode.DoubleRow
EXP_SHIFT = -2.0  # probs = exp(s/8 - 2); cancels in the softmax ratio

# cost-model pacing constants (ns)
SLOT = 1038.0  # one [128,1024] exp on ACT
C_SCORE = 426.0  # two [128,512] fp16 score matmuls
C_QKMM = 213.0
C_VUNIT = 480.0
C_DR = 214.0  # one hi+lo DoubleRow pair
C_PROJ = 640.0


def _build_module():
    nc = bacc.Bacc("TRN2", target_bir_lowering=False, debug=False, num_devices=NCORES)
    xT = nc.declare_dram_parameter("xT", [128, NDC, S], F16, isOutput=False)
    wqk = nc.declare_dram_parameter("wqk", [128, HPC, NDC, 128], F16, isOutput=False)
    wv = nc.declare_dram_parameter("wv", [128, NDC, VW], F16, isOutput=False)
    wo01 = nc.declare_dram_parameter("wo01", [128, D], F16, isOutput=False)
    wo2 = nc.declare_dram_parameter("wo2", [64, D], F16, isOutput=False)
    bqk = nc.declare_dram_parameter("bqk", [128, HPC], F32, isOutput=False)
    out = nc.declare_dram_parameter("out", [S, D], F16, isOutput=True)

    with TileContext(nc) as tc:
        _body(nc, tc, xT, wqk, wv, wo01, wo2, bqk, out)
    nc.compile()
    return nc


def _body(nc, tc, xT, wqk, wv, wo01, wo2, bqk, out):
    with (
        tc.tile_pool(name="persist", bufs=1) as P1,
        tc.tile_pool(name="work", bufs=4) as W2,
        tc.tile_pool(name="probs", bufs=3) as PR,
        # PSUM budget is 8 banks of [128, 512] fp32:
        #   ACC: one shared rotating pool for qk-proj, v-proj, ctx accum and
        #        out-proj tiles (4 banks)
        #   SPS: [128, 1024] score tiles, double-buffered (4 banks) -- pairs
        #        of key-chunks share one tile so exp runs 1024 wide
        tc.tile_pool(name="acc", bufs=4, space="PSUM") as ACC,
        tc.tile_pool(name="sps", bufs=2, space="PSUM") as SPS,
    ):
        xT_sb = P1.tile([128, NDC, S], F16, tag="xT")
        wqk_sb = P1.tile([128, HPC, NDC, 128], F16, tag="wqk")
        wv_sb = P1.tile([128, NDC, VW], F16, tag="wv")
        wo01_sb = P1.tile([128, D], F16, tag="wo01")
        wo2_sb = P1.tile([64, D], F16, tag="wo2")
        bqk_sb = P1.tile([128, HPC], F32, tag="bqk")
        ebias = P1.tile([128, 1], F32, tag="ebias")
        qT = [P1.tile([64, S], F16, tag=f"qT{h}", name=f"qT{h}") for h in range(HPC)]
        kT = [P1.tile([64, S], F16, tag=f"kT{h}", name=f"kT{h}") for h in range(HPC)]
        # v hi/lo fp8 tiles: per chunk-pair c and parity j, per head a
        # [v (64) | ones (64)] block (ones in hi, zeros in lo)
        vph = P1.tile([128, NPR, 2, HPC * 128], F8, tag="vph")
        vpl = P1.tile([128, NPR, 2, HPC * 128], F8, tag="vpl")

        # DMAs in first-needed order. Each dma_start pays ~1.3us of
        # serialized HWDGE/DGE overhead plus 0.9us sem latency, so batch
        # big -- except the first qk unit's dependencies (wqk head 0, xT
        # query-chunk 0 split per d-chunk) so PE can start early.
        nc.sync.dma_start(wqk_sb[:, 0, :, :], wqk[:, 0, :, :])
        nc.sync.dma_start(xT_sb[:, 0:3, 0:SQ], xT[:, 0:3, 0:SQ])
        nc.sync.dma_start(xT_sb[:, 3:6, 0:SQ], xT[:, 3:6, 0:SQ])
        nc.sync.dma_start(bqk_sb[:], bqk[:])
        for sc in range(1, NSQ):
            nc.sync.dma_start(
                xT_sb[:, :, sc * SQ:(sc + 1) * SQ], xT[:, :, sc * SQ:(sc + 1) * SQ]
            )
        nc.sync.dma_start(wv_sb[:], wv[:])
        nc.sync.dma_start(wqk_sb[:, 1:3, :, :], wqk[:, 1:3, :, :])
        nc.sync.dma_start(wo01_sb[:], wo01[:])
        nc.sync.dma_start(wo2_sb[:], wo2[:])
        nc.vector.memset(ebias[:], EXP_SHIFT)
        # ones columns next to each head's v-hi block (softmax denominator
        # trick); the v-lo ones-region must be zero (no double count)
        nc.gpsimd.memset(
            vph[:].rearrange("p c j (h m) -> p c j h m", m=128)[:, :, :, :, 64:128],
            1.0,
        )
        nc.gpsimd.memset(
            vpl[:].rearrange("p c j (h m) -> p c j h m", m=128)[:, :, :, :, 64:128],
            0.0,
        )

        # ---- emission scheduler state -------------------------------------
        emitted = set()  # readiness flags
        heap = []  # (deadline, seq, cost, flags_needed, fn)
        pending = []  # items whose flags aren't satisfied yet
        seq = count()
        credit = [0.0]
        slot = [0]

        def add(deadline, cost, fn, needs=()):
            item = (deadline, next(seq), cost, tuple(needs), fn)
            if all(f in emitted for f in item[3]):
                heapq.heappush(heap, item)
            else:
                pending.append(item)

        def refresh():
            still = []
            for item in pending:
                if all(f in emitted for f in item[3]):
                    heapq.heappush(heap, item)
                else:
                    still.append(item)
            pending[:] = still

        def pump(force_overdue=True):
            # emit overdue items regardless of budget, then spend credit
            while heap:
                deadline, _, cost, _, fn = heap[0]
                if deadline <= slot[0] and force_overdue:
                    pass
                elif credit[0] >= cost:
                    pass
                else:
                    break
                heapq.heappop(heap)
                fn()
                credit[0] -= cost
                refresh()

        # ---- work units ---------------------------------------------------
        qk_ps = {}

        def qk_mm(h, q, o):
            if o == 0:
                qk_ps[h, q] = ACC.tile([128, SQ], F32, tag="acc", name=f"qkps{h}_{q}")
            ps = qk_ps[h, q]
            nc.tensor.matmul(
                ps[:],
                wqk_sb[:, h, o, :],
                xT_sb[:, o, q * SQ:(q + 1) * SQ],
                start=(o == 0),
                stop=(o == NDC - 1),
            )
            if o == NDC - 1:
                nc.vector.tensor_tensor(
                    qT[h][:, q * SQ:(q + 1) * SQ],
                    ps[0:64, :],
                    bqk_sb[0:64, h:h + 1].to_broadcast([64, SQ]),
                    ADD,
                )
                # partition-shifted copy: psum rows 64..127 -> kT rows 0..63
                nc.vector.tensor_tensor(
                    kT[h][:, q * SQ:(q + 1) * SQ],
                    ps[64:128, :],
                    bqk_sb[64:128, h:h + 1].to_broadcast([64, SQ]),
                    ADD,
                )
                emitted.add(f"qk{h}_{q}")

        def v_unit(mk):
            # one key-chunk of v = xT.T @ [Wv_h0|Wv_h1|Wv_h2], split into
            # fp8 hi + lo (no bias: bv folds into bo on the host)
            c, j = mk // 2, mk % 2
            ps = ACC.tile([128, VW], F32, tag="acc", name=f"vps{mk}")
            for o in range(NDC):
                nc.tensor.matmul(
                    ps[:],
                    xT_sb[:, o, mk * 128:(mk + 1) * 128],
                    wv_sb[:, o, :],
                    start=(o == 0),
                    stop=(o == NDC - 1),
                )
            hi = vph[:, c, j, :].rearrange("p (h m) -> p h m", m=128)[:, :, 0:64]
            lo = vpl[:, c, j, :].rearrange("p (h m) -> p h m", m=128)[:, :, 0:64]
            psv = ps[:].rearrange("p (h m) -> p h m", m=64)
            nc.vector.tensor_copy(hi, psv)
            nc.vector.tensor_tensor(lo, psv, hi, SUB)
            emitted.add(f"v{mk}")

        probs_t = {}
        cps_t = {}
        ctxs = [
            (
                W2.tile([128, SQ], F16, tag="ctx01", name=f"c01_{sc}"),
                W2.tile([64, SQ], F16, tag="ctx2", name=f"c2_{sc}"),
            )
            for sc in range(NSQ)
        ]

        def dr_pair(b, c):
            h, sc = b // NSQ, b % NSQ
            if c == 0:
                cps_t[b] = ACC.tile([128, SQ], F32, tag="acc", name=f"cps{b}")
            cps = cps_t[b]
            pr = probs_t[b][:, c * 2 * SQ:(c + 1) * 2 * SQ].rearrange(
                "p (j n) -> p j n", j=2
            )
            nc.tensor.matmul(
                cps[:], vph[:, c, :, h * 128:(h + 1) * 128], pr,
                start=(c == 0), stop=False, perf_mode=DRM,
            )
            nc.tensor.matmul(
                cps[:], vpl[:, c, :, h * 128:(h + 1) * 128], pr,
                start=False, stop=(c == NPR - 1), perf_mode=DRM,
            )
            emitted.add(f"dr{b}_{c}")
            if c == NPR - 1:
                finish_block(b)

        def finish_block(b):
            # rows 0..63: unnormalized ctxT; rows 64..127: denominators
            h, sc = b // NSQ, b % NSQ
            cps = cps_t.pop(b)
            ctx01, ctx2 = ctxs[sc]
            last = b == NBLK - 1
            pieces = 4 if last else 1
            w = SQ // pieces
            for i in range(pieces):
                r = W2.tile([64, w], F32, tag="recip", name=f"r{b}_{i}")
                nc.vector.reciprocal(r[:], cps[64:128, i * w:(i + 1) * w])
                dst = ctx01[h * 64:(h + 1) * 64, :] if h < 2 else ctx2[:]
                nc.vector.tensor_tensor(
                    dst[:, i * w:(i + 1) * w], cps[0:64, i * w:(i + 1) * w],
                    r[:], MULT,
                )
                for ms in range(i * w // 128, (i + 1) * w // 128):
                    emitted.add(f"ctx{b}_{ms}")
            emitted.add(f"blk{b}")

        def proj_unit(sc, ms):
            # out[sc,ms] = ctx01.T @ Wo01 + ctx2.T @ Wo2, copied out as fp16
            ctx01, ctx2 = ctxs[sc]
            ot = W2.tile([128, D], F16, tag="out", name=f"ot{sc}_{ms}")
            for n0, nw in ((0, 512), (512, 256)):
                ops_t = ACC.tile([128, nw], F32, tag="acc", name=f"ops{sc}_{ms}_{n0}")
                nc.tensor.matmul(
                    ops_t[:], ctx01[:, ms * 128:(ms + 1) * 128],
                    wo01_sb[:, n0:n0 + nw], start=True, stop=False,
                )
                nc.tensor.matmul(
                    ops_t[:], ctx2[:, ms * 128:(ms + 1) * 128],
                    wo2_sb[:, n0:n0 + nw], start=False, stop=True,
                )
                if sc == NSQ - 1 and nw == 512:
                    # tail: the exp stream is over, ACT is idle -- route the
                    # wide copy through it so DVE only carries the narrow one
                    nc.scalar.activation(
                        ot[:, n0:n0 + nw], ops_t[:],
                        mybir.ActivationFunctionType.Copy,
                    )
                else:
                    nc.vector.tensor_copy(ot[:, n0:n0 + nw], ops_t[:])
            nc.sync.dma_start(
                out[(sc * 4 + ms) * 128:(sc * 4 + ms + 1) * 128, :], ot[:]
            )

        # ---- static work list ---------------------------------------------
        # qk unit (h, q): six matmuls; needed (kT side) by pair 2q of block
        # (h, 0) at slot h*32 + 2q, minus slack for the DVE bias-add.
        for h in range(HPC):
            for q in range(NSQ):
                d = h * 32 + 2 * q - 3 - (1 if q == 0 else 0)
                for o in range(NDC):
                    add(d, C_QKMM, lambda h=h, q=q, o=o: qk_mm(h, q, o))
        # v units: needed by the lagged DoubleRow of block 0 onward
        for mk in range(NSK):
            add(6 + mk, C_VUNIT, lambda mk=mk: v_unit(mk))
        # DoubleRow probs@V pairs: blocks 0..3 (head-0 phase) may lag up to
        # the probs-buffer deadline (3 buffers); later blocks pair-lag so
        # ctx/proj complete in-phase.
        for b in range(NBLK):
            for c in range(NPR):
                d = 8 * (b + 3) - 1 if b < NSQ else 8 * b + c + 2
                add(d, C_DR, lambda b=b, c=c: dr_pair(b, c),
                    needs=(f"exp{b}_{c}", f"v{2 * c}", f"v{2 * c + 1}"))
        # out projections: after the h2 block of sc completes ctx2 (and
        # ctx01 long before). sc=3 lands in the tail, per-ms pipelined.
        for sc in range(NSQ):
            b2 = 2 * NSQ + sc
            for ms in range(4):
                add(8 * (b2 + 1) + 2 * ms, C_PROJ,
                    lambda sc=sc, ms=ms: proj_unit(sc, ms),
                    needs=(f"blk{NSQ + sc}", f"ctx{b2}_{ms}"))

        # PE warmup: the cost model's p-state ramp needs ~3us of sustained
        # matmul activity for full clock; the first real matmuls wait on DMA
        # anyway, so burn the wait on narrow dummy matmuls (128-wide: cheap
        # to preempt) that bridge the gap until the first xT slab lands.
        warm = P1.tile([64, 512], F16, tag="warm")
        nc.vector.memset(warm[:].bitcast(F32), 0.0)
        wps = ACC.tile([128, 128], F32, tag="acc", name="warmps")
        for _ in range(28):
            nc.tensor.matmul(
                wps[:], warm[:, 0:128], warm[:, 0:128], start=True, stop=True
            )
        # pre-load the ACT exp table set during the same dead time
        wact = P1.tile([64, 1], F16, tag="wact")
        nc.scalar.activation(wact[:], warm[:, 0:2].bitcast(F32), EXP, scale=0.125)

        # ---- slot loop: one exp per slot, budget-paced fillers ------------
        for b in range(NBLK):
            h, sc = b // NSQ, b % NSQ
            probs_t[b] = PR.tile([128, NSK * SQ], F8, tag="probs", name=f"pr{b}")
            for j in range(NPR):
                pump()  # overdue first (qk deps for these scores)
                sps = SPS.tile([128, 2 * SQ], F32, tag="sps", name=f"sps{b}_{j}")
                for half in range(2):
                    mk = 2 * j + half
                    nc.tensor.matmul(
                        sps[:, half * SQ:(half + 1) * SQ],
                        kT[h][:, mk * 128:(mk + 1) * 128],
                        qT[h][:, sc * SQ:(sc + 1) * SQ],
                        start=True,
                        stop=True,
                    )
                nc.scalar.activation(
                    probs_t[b][:, j * 2 * SQ:(j + 1) * 2 * SQ], sps[:], EXP,
                    scale=0.125, bias=ebias[:],
                )
                emitted.add(f"exp{b}_{j}")
                refresh()
                credit[0] = min(credit[0] + SLOT - C_SCORE, 4 * SLOT)
                pump()
                slot[0] += 1
        # tail: drain everything left (last block's DR pairs, ctx, proj sc=3)
        credit[0] = 1e9
        while heap or pending:
            n0 = len(heap) + len(pending)
            pump()
            if len(heap) + len(pending) == n0:
                raise RuntimeError(
                    f"scheduler deadlock: {len(heap)} heap / {len(pending)} pending"
                )


_CACHE = {}


def _get_module():
    if "nc" not in _CACHE:
        _CACHE["nc"] = _build_module()
    return _CACHE["nc"]


def make_in_maps(x, Wq, Wk, Wv, bq, bk, bv, Wo):
    f16 = np.float16
    in_maps = []
    for c in range(NCORES):
        b = c // CORES_PER_BATCH
        hh = [HPC * (c % CORES_PER_BATCH) + i for i in range(HPC)]
        # xT pre-tiled to [128, 6, 2048]: partition p, d-chunk o, seq s
        xt = x[b].T.reshape(NDC, 128, S).transpose(1, 0, 2)
        # wqk pre-tiled to [128, 3, 6, 128]
        wqk = np.stack(
            [np.concatenate([Wq[h], Wk[h]], axis=1) for h in hh]
        )  # [3, 768, 128]
        wqk = wqk.reshape(HPC, NDC, 128, 128).transpose(2, 0, 1, 3)
        # wv pre-tiled to [128, 6, 192]
        wv_stack = np.concatenate([Wv[h] for h in hh], axis=1)  # [768, 192]
        wv_stack = wv_stack.reshape(NDC, 128, VW).transpose(1, 0, 2)
        in_maps.append({
            "xT": np.ascontiguousarray(xt).astype(f16),
            "wqk": np.ascontiguousarray(wqk).astype(f16),
            "wv": np.ascontiguousarray(wv_stack).astype(f16),
            "wo01": np.ascontiguousarray(
                Wo[hh[0] * DH:(hh[0] + 2) * DH, :]
            ).astype(f16),
            "wo2": np.ascontiguousarray(
                Wo[hh[2] * DH:(hh[2] + 1) * DH, :]
            ).astype(f16),
            "bqk": np.ascontiguousarray(
                np.stack([np.concatenate([bq[h], bk[h]]) for h in hh], axis=1)
            ).astype(np.float32),
        })
    return in_maps


def gather(results, bv, Wo, bo):
    # ctx_h = softmax(scores) @ v_nobias + bv_h, so the bv contribution to
    # the output is a constant row: sum_h bv_h @ Wo_h, folded into bo here.
    bo_eff = bo.astype(np.float64) + bv.reshape(-1).astype(np.float64) @ Wo.astype(
        np.float64
    )
    out = np.empty((B, S, D), np.float32)
    for b in range(B):
        acc = results[b * CORES_PER_BATCH]["out"].astype(np.float64, copy=True)
        for c in range(b * CORES_PER_BATCH + 1, (b + 1) * CORES_PER_BATCH):
            acc += results[c]["out"].astype(np.float64)
        out[b] = (acc + bo_eff[None, :]).astype(np.float32)
    return out


def kernel(x, Wq, Wk, Wv, bq, bk, bv, Wo, bo, c=0, **_unused):
    x, Wq, Wk, Wv, bq, bk, bv, Wo, bo = (
        np.asarray(a, np.float32) for a in (x, Wq, Wk, Wv, bq, bk, bv, Wo, bo)
    )
    nc = _get_module()
    in_maps = make_in_maps(x, Wq, Wk, Wv, bq, bk, bv, Wo)
    res = run_bass_kernel_spmd(nc, in_maps, list(range(NCORES)))
    return gather(res.results, bv, Wo, bo)


# revision 11
# speedup vs baseline: 1.2473x; 1.0053x over previous
"""Multi-head attention kernel for Trainium2, sharded over 8 NeuronCores.

Sharding: data parallel over batch (B=2 -> 4 cores each) x tensor parallel
over heads (12 heads -> 3 heads per core). Each core computes QKV projections,
attention, and a partial output projection for its 3 heads; the per-head
partial output projections are summed on the host (the all-reduce of the
tensor-parallel hint, done during the gather step) and the output bias added.

Design (per core):
  - All matmul operands are fp16 (x, W*, q, k, ctx, Wo) -- halves input DMA
    vs fp32 at full matmul rate; accumulations stay fp32 in PSUM. Partial
    outputs return as fp16 (summed in fp32 on the host).
  - x arrives pre-transposed and pre-tiled as xT [128, 6, 2048] so the
    contraction dim (d) sits on SBUF partitions for all QKV matmuls.
  - q and k are produced head-by-head directly in transposed form via a
    stacked weight [Wq_h | Wk_h]; scores are computed transposed
    (scoresT [s_k, s_q]) so the softmax probabilities feed the probs@V
    matmul with no transpose.
  - probs are written by the scalar engine as fp8e4m3 = exp(s/8 - 2); the
    -2 shift keeps the distribution inside e4m3's range and cancels exactly
    in the softmax ratio.
  - probs@V runs in fp8 DoubleRow perf mode (two 128-deep contraction
    chunks per matmul at 2x rate). v rides as an fp8 hi+lo pair
    (lo = fp8(v - fp8(v))) accumulated into the same PSUM group, so v keeps
    ~fp16 accuracy while both DoubleRow operands are fp8.
  - softmax denominators ride for free: the v-hi operand carries a block of
    ones columns (zeros in v-lo), so rows 64..127 of the probs@V
    accumulation are the per-query sums of the same quantized probs --
    quantization partially cancels in the ratio.
  - the v bias never touches the device: ctx_h = sum(p v)/sum(p) + bv_h, so
    sum_h bv_h @ Wo_h folds into the output bias on the host.
  - with probs@V halved by DoubleRow the kernel is ACT(exp)-bound, so the
    emission scheduler paces one [128,1024] exp per ~1038ns slot and packs
    all other PE work (qk/v/out projections, lagged DoubleRow matmuls) into
    the leftover budget of each slot, deadline-ordered. Blocks run
    head-major so each phase only needs its own head's q/k projections.
"""

import heapq
from itertools import count

import numpy as np

import concourse.mybir as mybir
from concourse import bacc
from concourse.tile import TileContext
from concourse.bass_utils import run_bass_kernel_spmd

H, D, DH = 12, 768, 64
B, S = 2, 2048
NCORES = 8
CORES_PER_BATCH = 4
HPC = 3  # heads per core
SQ = 512  # query-chunk width
NSQ = S // SQ  # 4
NSK = S // 128  # 16 key chunks
NPR = NSK // 2  # 8 key-chunk pairs
NDC = D // 128  # 6 contraction chunks
VW = HPC * DH  # 192 v columns per core
NBLK = HPC * NSQ  # 12 attention blocks, head-major: b = h*NSQ + sc

F32 = mybir.dt.float32
F16 = mybir.dt.float16
F8 = mybir.dt.float8e4
ADD = mybir.AluOpType.add
SUB = mybir.AluOpType.subtract
MULT = mybir.AluOpType.mult
EXP = mybir.ActivationFunctionType.Exp
DRM = mybir.MatmulPerfMode.DoubleRow
EXP_SHIFT = -2.0  # probs = exp(s/8 - 2); cancels in the softmax ratio

# cost-model pacing constants (ns)
SLOT = 1038.0  # one [128,1024] exp on ACT
C_SCORE = 426.0  # two [128,512] fp16 score matmuls
C_QKMM = 213.0
C_VUNIT = 480.0
C_DR = 214.0  # one hi+lo DoubleRow pair
C_PROJ = 640.0


def _build_module():
    nc = bacc.Bacc("TRN2", target_bir_lowering=False, debug=False, num_devices=NCORES)
    xT = nc.declare_dram_parameter("xT", [128, NDC, S], F16, isOutput=False)
    wqk = nc.declare_dram_parameter("wqk", [128, HPC, NDC, 128], F16, isOutput=False)
    wv = nc.declare_dram_parameter("wv", [128, NDC, VW], F16, isOutput=False)
    wo01 = nc.declare_dram_parameter("wo01", [128, D], F16, isOutput=False)
    wo2 = nc.declare_dram_parameter("wo2", [64, D], F16, isOutput=False)
    bqk = nc.declare_dram_parameter("bqk", [128, HPC], F32, isOutput=False)
    out = nc.declare_dram_parameter("out", [S, D], F16, isOutput=True)

    with TileContext(nc) as tc:
        _body(nc, tc, xT, wqk, wv, wo01, wo2, bqk, out)
    nc.compile()
    return nc


def _body(nc, tc, xT, wqk, wv, wo01, wo2, bqk, out):
    with (
        tc.tile_pool(name="persist", bufs=1) as P1,
        tc.tile_pool(name="work", bufs=4) as W2,
        tc.tile_pool(name="probs", bufs=3) as PR,
        # PSUM budget is 8 banks of [128, 512] fp32:
        #   ACC: one shared rotating pool for qk-proj, v-proj, ctx accum and
        #        out-proj tiles (4 banks)
        #   SPS: [128, 1024] score tiles, double-buffered (4 banks) -- pairs
        #        of key-chunks share one tile so exp runs 1024 wide
        tc.tile_pool(name="acc", bufs=4, space="PSUM") as ACC,
        tc.tile_pool(name="sps", bufs=2, space="PSUM") as SPS,
    ):
        xT_sb = P1.tile([128, NDC, S], F16, tag="xT")
        wqk_sb = P1.tile([128, HPC, NDC, 128], F16, tag="wqk")
        wv_sb = P1.tile([128, NDC, VW], F16, tag="wv")
        wo01_sb = P1.tile([128, D], F16, tag="wo01")
        wo2_sb = P1.tile([64, D], F16, tag="wo2")
        bqk_sb = P1.tile([128, HPC], F32, tag="bqk")
        ebias = P1.tile([128, 1], F32, tag="ebias")
        qT = [P1.tile([64, S], F16, tag=f"qT{h}", name=f"qT{h}") for h in range(HPC)]
        kT = [P1.tile([64, S], F16, tag=f"kT{h}", name=f"kT{h}") for h in range(HPC)]
        # v hi/lo fp8 tiles: per chunk-pair c and parity j, per head a
        # [v (64) | ones (64)] block (ones in hi, zeros in lo)
        vph = P1.tile([128, NPR, 2, HPC * 128], F8, tag="vph")
        vpl = P1.tile([128, NPR, 2, HPC * 128], F8, tag="vpl")

        # DMAs in first-needed order. Each dma_start pays ~1.3us of
        # serialized HWDGE/DGE overhead plus 0.9us sem latency, so batch
        # big -- except the first qk unit's dependencies (wqk head 0, xT
        # query-chunk 0 split per d-chunk) so PE can start early.
        nc.sync.dma_start(wqk_sb[:, 0, :, :], wqk[:, 0, :, :])
        nc.sync.dma_start(xT_sb[:, 0:3, 0:SQ], xT[:, 0:3, 0:SQ])
        nc.sync.dma_start(xT_sb[:, 3:6, 0:SQ], xT[:, 3:6, 0:SQ])
        nc.sync.dma_start(bqk_sb[:], bqk[:])
        for sc in range(1, NSQ):
            nc.sync.dma_start(
                xT_sb[:, :, sc * SQ:(sc + 1) * SQ], xT[:, :, sc * SQ:(sc + 1) * SQ]
            )
        nc.sync.dma_start(wv_sb[:], wv[:])
        nc.sync.dma_start(wqk_sb[:, 1:3, :, :], wqk[:, 1:3, :, :])
        nc.sync.dma_start(wo01_sb[:], wo01[:])
        nc.sync.dma_start(wo2_sb[:], wo2[:])
        nc.vector.memset(ebias[:], EXP_SHIFT)
        # ones columns next to each head's v-hi block (softmax denominator
        # trick); the v-lo ones-region must be zero (no double count)
        nc.gpsimd.memset(
            vph[:].rearrange("p c j (h m) -> p c j h m", m=128)[:, :, :, :, 64:128],
            1.0,
        )
        nc.gpsimd.memset(
            vpl[:].rearrange("p c j (h m) -> p c j h m", m=128)[:, :, :, :, 64:128],
            0.0,
        )

        # ---- emission scheduler state -------------------------------------
        emitted = set()  # readiness flags
        heap = []  # (deadline, seq, cost, flags_needed, fn)
        pending = []  # items whose flags aren't satisfied yet
        seq = count()
        credit = [0.0]
        slot = [0]

        def add(deadline, cost, fn, needs=()):
            item = (deadline, next(seq), cost, tuple(needs), fn)
            if all(f in emitted for f in item[3]):
                heapq.heappush(heap, item)
            else:
                pending.append(item)

        def refresh():
            still = []
            for item in pending:
                if all(f in emitted for f in item[3]):
                    heapq.heappush(heap, item)
                else:
                    still.append(item)
            pending[:] = still

        def pump(force_overdue=True):
            # emit overdue items regardless of budget, then spend credit
            while heap:
                deadline, _, cost, _, fn = heap[0]
                if deadline <= slot[0] and force_overdue:
                    pass
                elif credit[0] >= cost:
                    pass
                else:
                    break
                heapq.heappop(heap)
                fn()
                credit[0] -= cost
                refresh()

        # ---- work units ---------------------------------------------------
        qk_ps = {}

        def qk_mm(h, q, o):
            if o == 0:
                qk_ps[h, q] = ACC.tile([128, SQ], F32, tag="acc", name=f"qkps{h}_{q}")
            ps = qk_ps[h, q]
            nc.tensor.matmul(
                ps[:],
                wqk_sb[:, h, o, :],
                xT_sb[:, o, q * SQ:(q + 1) * SQ],
                start=(o == 0),
                stop=(o == NDC - 1),
            )
            if o == NDC - 1:
                nc.vector.tensor_tensor(
                    qT[h][:, q * SQ:(q + 1) * SQ],
                    ps[0:64, :],
                    bqk_sb[0:64, h:h + 1].to_broadcast([64, SQ]),
                    ADD,
                )
                # partition-shifted copy: psum rows 64..127 -> kT rows 0..63
                nc.vector.tensor_tensor(
                    kT[h][:, q * SQ:(q + 1) * SQ],
                    ps[64:128, :],
                    bqk_sb[64:128, h:h + 1].to_broadcast([64, SQ]),
                    ADD,
                )
                emitted.add(f"qk{h}_{q}")

        def v_unit(mk):
            # one key-chunk of v = xT.T @ [Wv_h0|Wv_h1|Wv_h2], split into
            # fp8 hi + lo (no bias: bv folds into bo on the host)
            c, j = mk // 2, mk % 2
            ps = ACC.tile([128, VW], F32, tag="acc", name=f"vps{mk}")
            for o in range(NDC):
                nc.tensor.matmul(
                    ps[:],
                    xT_sb[:, o, mk * 128:(mk + 1) * 128],
                    wv_sb[:, o, :],
                    start=(o == 0),
                    stop=(o == NDC - 1),
                )
            hi = vph[:, c, j, :].rearrange("p (h m) -> p h m", m=128)[:, :, 0:64]
            lo = vpl[:, c, j, :].rearrange("p (h m) -> p h m", m=128)[:, :, 0:64]
            psv = ps[:].rearrange("p (h m) -> p h m", m=64)
            nc.vector.tensor_copy(hi, psv)
            nc.vector.tensor_tensor(lo, psv, hi, SUB)
            emitted.add(f"v{mk}")

        probs_t = {}
        cps_t = {}
        ctxs = [
            (
                W2.tile([128, SQ], F16, tag="ctx01", name=f"c01_{sc}"),
                W2.tile([64, SQ], F16, tag="ctx2", name=f"c2_{sc}"),
            )
            for sc in range(NSQ)
        ]

        def dr_pair(b, c):
            h, sc = b // NSQ, b % NSQ
            if c == 0:
                cps_t[b] = ACC.tile([128, SQ], F32, tag="acc", name=f"cps{b}")
            cps = cps_t[b]
            pr = probs_t[b][:, c * 2 * SQ:(c + 1) * 2 * SQ].rearrange(
                "p (j n) -> p j n", j=2
            )
            nc.tensor.matmul(
                cps[:], vph[:, c, :, h * 128:(h + 1) * 128], pr,
                start=(c == 0), stop=False, perf_mode=DRM,
            )
            nc.tensor.matmul(
                cps[:], vpl[:, c, :, h * 128:(h + 1) * 128], pr,
                start=False, stop=(c == NPR - 1), perf_mode=DRM,
            )
            emitted.add(f"dr{b}_{c}")
            if c == NPR - 1:
                finish_block(b)

        def finish_block(b):
            # rows 0..63: unnormalized ctxT; rows 64..127: denominators
            h, sc = b // NSQ, b % NSQ
            cps = cps_t.pop(b)
            ctx01, ctx2 = ctxs[sc]
            last = b == NBLK - 1
            pieces = 4 if last else 1
            w = SQ // pieces
            for i in range(pieces):
                r = W2.tile([64, w], F32, tag="recip", name=f"r{b}_{i}")
                nc.vector.reciprocal(r[:], cps[64:128, i * w:(i + 1) * w])
                dst = ctx01[h * 64:(h + 1) * 64, :] if h < 2 else ctx2[:]
                nc.vector.tensor_tensor(
                    dst[:, i * w:(i + 1) * w], cps[0:64, i * w:(i + 1) * w],
                    r[:], MULT,
                )
                for ms in range(i * w // 128, (i + 1) * w // 128):
                    emitted.add(f"ctx{b}_{ms}")
            emitted.add(f"blk{b}")

        def proj_unit(sc, ms):
            # out[sc,ms] = ctx01.T @ Wo01 + ctx2.T @ Wo2, copied out as fp16
            ctx01, ctx2 = ctxs[sc]
            tail = sc == NSQ - 1
            row = (sc * 4 + ms) * 128
            ot = W2.tile([128, D], F16, tag="out", name=f"ot{sc}_{ms}")
            for n0, nw in ((0, 512), (512, 256)):
                ops_t = ACC.tile([128, nw], F32, tag="acc", name=f"ops{sc}_{ms}_{n0}")
                nc.tensor.matmul(
                    ops_t[:], ctx01[:, ms * 128:(ms + 1) * 128],
                    wo01_sb[:, n0:n0 + nw], start=True, stop=False,
                )
                nc.tensor.matmul(
                    ops_t[:], ctx2[:, ms * 128:(ms + 1) * 128],
                    wo2_sb[:, n0:n0 + nw], start=False, stop=True,
                )
                if tail and ms % 2 == 0:
                    # tail: the exp stream is over, ACT is idle -- alternate
                    # whole-ms copies between ACT and DVE to halve the
                    # serial copy chain
                    nc.scalar.activation(
                        ot[:, n0:n0 + nw], ops_t[:],
                        mybir.ActivationFunctionType.Copy,
                    )
                else:
                    nc.vector.tensor_copy(ot[:, n0:n0 + nw], ops_t[:])
                if tail:
                    # per-piece DMA so the last transfer is small
                    nc.sync.dma_start(
                        out[row:row + 128, n0:n0 + nw], ot[:, n0:n0 + nw]
                    )
            if not tail:
                nc.sync.dma_start(out[row:row + 128, :], ot[:])

        # ---- static work list ---------------------------------------------
        # qk unit (h, q): six matmuls; needed (kT side) by pair 2q of block
        # (h, 0) at slot h*32 + 2q, minus slack for the DVE bias-add.
        for h in range(HPC):
            for q in range(NSQ):
                d = h * 32 + 2 * q - 3 - (1 if q == 0 else 0)
                for o in range(NDC):
                    add(d, C_QKMM, lambda h=h, q=q, o=o: qk_mm(h, q, o))
        # v units: needed by the lagged DoubleRow of block 0 onward
        for mk in range(NSK):
            add(6 + mk, C_VUNIT, lambda mk=mk: v_unit(mk))
        # DoubleRow probs@V pairs: blocks 0..3 (head-0 phase) may lag up to
        # the probs-buffer deadline (3 buffers); later blocks pair-lag so
        # ctx/proj complete in-phase.
        for b in range(NBLK):
            for c in range(NPR):
                d = 8 * (b + 3) - 1 if b < NSQ else 8 * b + c + 2
                add(d, C_DR, lambda b=b, c=c: dr_pair(b, c),
                    needs=(f"exp{b}_{c}", f"v{2 * c}", f"v{2 * c + 1}"))
        # out projections: after the h2 block of sc completes ctx2 (and
        # ctx01 long before). sc=3 lands in the tail, per-ms pipelined.
        for sc in range(NSQ):
            b2 = 2 * NSQ + sc
            for ms in range(4):
                add(8 * (b2 + 1) + 2 * ms, C_PROJ,
                    lambda sc=sc, ms=ms: proj_unit(sc, ms),
                    needs=(f"blk{NSQ + sc}", f"ctx{b2}_{ms}"))

        # PE warmup: the cost model's p-state ramp needs ~3us of sustained
        # matmul activity for full clock; the first real matmuls wait on DMA
        # anyway, so burn the wait on narrow dummy matmuls (128-wide: cheap
        # to preempt) that bridge the gap until the first xT slab lands.
        warm = P1.tile([64, 512], F16, tag="warm")
        nc.vector.memset(warm[:].bitcast(F32), 0.0)
        wps = ACC.tile([128, 128], F32, tag="acc", name="warmps")
        for _ in range(28):
            nc.tensor.matmul(
                wps[:], warm[:, 0:128], warm[:, 0:128], start=True, stop=True
            )
        # pre-load the ACT exp table set during the same dead time
        wact = P1.tile([64, 1], F16, tag="wact")
        nc.scalar.activation(wact[:], warm[:, 0:2].bitcast(F32), EXP, scale=0.125)

        # ---- slot loop: one exp per slot, budget-paced fillers. Scores are
        # emitted one slot ahead of their exp so each slot's PE stream leads
        # with the matmuls ACT is about to need (filler overruns then only
        # delay fillers, not the exp cadence).
        pairs = [(b, j) for b in range(NBLK) for j in range(NPR)]
        sps_t = {}

        def emit_exp(p):
            b, j = pairs[p]
            nc.scalar.activation(
                probs_t[b][:, j * 2 * SQ:(j + 1) * 2 * SQ], sps_t.pop(p)[:], EXP,
                scale=0.125, bias=ebias[:],
            )
            emitted.add(f"exp{b}_{j}")
            refresh()

        for p, (b, j) in enumerate(pairs):
            h, sc = b // NSQ, b % NSQ
            if j == 0:
                probs_t[b] = PR.tile([128, NSK * SQ], F8, tag="probs", name=f"pr{b}")
            pump()  # overdue first (qk deps for these scores)
            sps = sps_t[p] = SPS.tile(
                [128, 2 * SQ], F32, tag="sps", name=f"sps{b}_{j}"
            )
            for half in range(2):
                mk = 2 * j + half
                nc.tensor.matmul(
                    sps[:, half * SQ:(half + 1) * SQ],
                    kT[h][:, mk * 128:(mk + 1) * 128],
                    qT[h][:, sc * SQ:(sc + 1) * SQ],
                    start=True,
                    stop=True,
                )
            if p > 0:
                emit_exp(p - 1)
            credit[0] = min(credit[0] + SLOT - C_SCORE, 4 * SLOT)
            pump()
            slot[0] += 1
        emit_exp(len(pairs) - 1)
        # tail: drain everything left (last block's DR pairs, ctx, proj sc=3)
        credit[0] = 1e9
        while heap or pending:
            n0 = len(heap) + len(pending)
            pump()
            if len(heap) + len(pending) == n0:
                raise RuntimeError(
                    f"scheduler deadlock: {len(heap)} heap / {len(pending)} pending"
                )


_CACHE = {}


def _get_module():
    if "nc" not in _CACHE:
        _CACHE["nc"] = _build_module()
    return _CACHE["nc"]


def make_in_maps(x, Wq, Wk, Wv, bq, bk, bv, Wo):
    f16 = np.float16
    in_maps = []
    for c in range(NCORES):
        b = c // CORES_PER_BATCH
        hh = [HPC * (c % CORES_PER_BATCH) + i for i in range(HPC)]
        # xT pre-tiled to [128, 6, 2048]: partition p, d-chunk o, seq s
        xt = x[b].T.reshape(NDC, 128, S).transpose(1, 0, 2)
        # wqk pre-tiled to [128, 3, 6, 128]
        wqk = np.stack(
            [np.concatenate([Wq[h], Wk[h]], axis=1) for h in hh]
        )  # [3, 768, 128]
        wqk = wqk.reshape(HPC, NDC, 128, 128).transpose(2, 0, 1, 3)
        # wv pre-tiled to [128, 6, 192]
        wv_stack = np.concatenate([Wv[h] for h in hh], axis=1)  # [768, 192]
        wv_stack = wv_stack.reshape(NDC, 128, VW).transpose(1, 0, 2)
        in_maps.append({
            "xT": np.ascontiguousarray(xt).astype(f16),
            "wqk": np.ascontiguousarray(wqk).astype(f16),
            "wv": np.ascontiguousarray(wv_stack).astype(f16),
            "wo01": np.ascontiguousarray(
                Wo[hh[0] * DH:(hh[0] + 2) * DH, :]
            ).astype(f16),
            "wo2": np.ascontiguousarray(
                Wo[hh[2] * DH:(hh[2] + 1) * DH, :]
            ).astype(f16),
            "bqk": np.ascontiguousarray(
                np.stack([np.concatenate([bq[h], bk[h]]) for h in hh], axis=1)
            ).astype(np.float32),
        })
    return in_maps


def gather(results, bv, Wo, bo):
    # ctx_h = softmax(scores) @ v_nobias + bv_h, so the bv contribution to
    # the output is a constant row: sum_h bv_h @ Wo_h, folded into bo here.
    bo_eff = bo.astype(np.float64) + bv.reshape(-1).astype(np.float64) @ Wo.astype(
        np.float64
    )
    out = np.empty((B, S, D), np.float32)
    for b in range(B):
        acc = results[b * CORES_PER_BATCH]["out"].astype(np.float64, copy=True)
        for c in range(b * CORES_PER_BATCH + 1, (b + 1) * CORES_PER_BATCH):
            acc += results[c]["out"].astype(np.float64)
        out[b] = (acc + bo_eff[None, :]).astype(np.float32)
    return out


def kernel(x, Wq, Wk, Wv, bq, bk, bv, Wo, bo, c=0, **_unused):
    x, Wq, Wk, Wv, bq, bk, bv, Wo, bo = (
        np.asarray(a, np.float32) for a in (x, Wq, Wk, Wv, bq, bk, bv, Wo, bo)
    )
    nc = _get_module()
    in_maps = make_in_maps(x, Wq, Wk, Wv, bq, bk, bv, Wo)
    res = run_bass_kernel_spmd(nc, in_maps, list(range(NCORES)))
    return gather(res.results, bv, Wo, bo)
